# revision 1
# baseline (speedup 1.0000x reference)
"""Trainium2 Bass kernel: causal multi-head attention (B=4,S=2048,D=1024,H=16).

Sharding (8 cores, no collectives): core c -> batch b=c//2, q-half h=c%2.
Each core computes all 16 heads for 8 interleaved query tiles of 128 rows
(abs q-tile t = 2*j + h for local slot j), plus full K/V for its batch,
and the full fc_out for its own query rows.  The host scatters the 8
per-core [1024,1024] outputs back into [4,2048,1024].

Device pipeline per core (all matmuls bf16, f32 accumulation):
  P1: Q/K/V projections (stationary x^T blocks, moving per-head weights),
      PSUM->SBUF cast + bias, DMA-xbar transposes to build Q^T/K^T.
  P2: per (head, k-tile): scores^T = K^T.T @ Q^T -> PSUM, exp via ScalarE
      (scale=1/8 folded in), 0/1 mask multiply on "mixed" tiles only,
      out^T accumulation with ones-augmented V (row 64 = softmax denom).
      Normalization by the reciprocal of the denominator at head end.
  P3: fc_out = concat^T.T @ Wo + bo for the local query rows.

The program is specialized at build time to the mask's block structure
(skip all-zero blocks / skip masking on all-ones blocks); this is computed
from the actual mask input, so it stays correct for any mask.
"""

import os
import numpy as np
import ml_dtypes

import concourse.bass as bass
import concourse.mybir as mybir
import concourse.tile as tile
from concourse import bacc
from concourse.bass_utils import run_bass_kernel_spmd

B, S, D, H, HD = 4, 2048, 1024, 16, 64
N_CORES = 8
ST = 128               # tile edge (partition size)
NKT = S // ST          # 16 key tiles
NJ = 8                 # local query slots per core (8*128 = 1024 rows)
NDC = D // ST          # 8 contraction chunks
NG = H // 2            # 8 head pairs (2 heads packed per 128 partitions)

F32 = mybir.dt.float32
BF16 = mybir.dt.bfloat16


def _classify(mask: np.ndarray):
    """Block structure of the mask, unioned over the two q-halves.

    Returns (cls[NJ][NKT] in {0 skip,1 full,2 mixed}, mixed list [(j,k)]).
    """
    cls = np.zeros((NJ, NKT), dtype=int)
    for j in range(NJ):
        for k in range(NKT):
            blocks = [
                mask[(2 * j + h) * ST:(2 * j + h + 1) * ST, k * ST:(k + 1) * ST]
                for h in (0, 1)
            ]
            if all((b != 0).all() for b in blocks):
                cls[j, k] = 1
            elif all((b == 0).all() for b in blocks):
                cls[j, k] = 0
            else:
                cls[j, k] = 2
        # close interior holes so every slot's computed k-range is contiguous
        nz = np.nonzero(cls[j])[0]
        if len(nz):
            for k in range(nz[0], nz[-1] + 1):
                if cls[j, k] == 0:
                    cls[j, k] = 2
    mixed = [(j, k) for j in range(NJ) for k in range(NKT) if cls[j, k] == 2]
    return cls, mixed


def _build(cls, mixed, n_maskt):
    """Build the (uniform, SPMD) Bass program for one core's shard."""
    nc = bacc.Bacc("TRN2", target_bir_lowering=False, debug=False,
                   num_devices=N_CORES)

    x_d = nc.dram_tensor("x", [S, D], F32, kind="ExternalInput")
    xq_d = nc.dram_tensor("xq", [NJ * ST, D], F32, kind="ExternalInput")
    wq_d = nc.dram_tensor("wq", [H, D, HD], F32, kind="ExternalInput")
    wk_d = nc.dram_tensor("wk", [H, D, HD], F32, kind="ExternalInput")
    wv_d = nc.dram_tensor("wv", [H, D, HD], F32, kind="ExternalInput")
    wo_d = nc.dram_tensor("wo", [D, D], F32, kind="ExternalInput")
    bq_d = nc.dram_tensor("bq", [H, HD], F32, kind="ExternalInput")
    bk_d = nc.dram_tensor("bk", [H, HD], F32, kind="ExternalInput")
    bv_d = nc.dram_tensor("bv", [H, HD], F32, kind="ExternalInput")
    bo_d = nc.dram_tensor("bo", [D], F32, kind="ExternalInput")
    mt_d = nc.dram_tensor("maskt", [n_maskt, ST, ST], BF16, kind="ExternalInput")
    out_d = nc.dram_tensor("out", [NJ * ST, D], F32, kind="ExternalOutput")

    mixed_idx = {jk: i for i, jk in enumerate(mixed)}
    # per-k slot spans and per-slot k ranges
    slots_k = [[j for j in range(NJ) if cls[j, k]] for k in range(NKT)]
    kfirst = {}
    klast = {}
    for j in range(NJ):
        ks = [k for k in range(NKT) if cls[j, k]]
        if ks:
            kfirst[j], klast[j] = ks[0], ks[-1]

    NB = NJ // 4  # PSUM 512-col banks per po tile (2)
    NSG = NKT // 4  # 4 s-groups of 512 rows
    bank_slots = [[j for j in range(4 * b_, 4 * b_ + 4) if j in kfirst]
                  for b_ in range(NB)]
    bklast = {b_: max(klast[j] for j in bank_slots[b_])
              for b_ in range(NB) if bank_slots[b_]}
    bank_fast = {b_: len({kfirst[j] for j in bank_slots[b_]}) == 1
                 for b_ in range(NB) if bank_slots[b_]}

    from concourse.masks import make_identity

    with tile.TileContext(nc) as tc:
        with (
            tc.tile_pool(name="persist", bufs=1) as pp,      # lives whole kernel
        ):
            # ---- persistent SBUF tensors -------------------------------
            kt_t = [pp.tile([ST, S], BF16, name=f"ktg{g}", tag=f"ktg{g}")
                    for g in range(NG)]
            qt_t = [pp.tile([ST, NJ * ST], BF16, name=f"qtg{g}", tag=f"qtg{g}")
                    for g in range(NG)]
            vb = pp.tile([ST, NKT, H, HD + 1], BF16, name="vb", tag="vb")
            bob = pp.tile([ST, D], F32, name="bob", tag="bob")
            ident = pp.tile([ST, ST], BF16, name="ident", tag="ident")

            nc.vector.memset(vb[:, :, :, HD:HD + 1], 1.0)
            make_identity(nc, ident[:, :])
            bo_ap = bo_d.ap()
            nc.sync.dma_start(
                bob[:, :],
                bass.AP(tensor=bo_ap.tensor, offset=bo_ap.offset,
                        ap=[[0, ST]] + list(bo_ap.ap)))

            def load_bias_pair(pool, bias_d, name):
                # [128, NG] f32: partition = (h%2)*64+e, column = pair idx
                t = pool.tile([ST, NG], F32, name=name, tag=name, bufs=1)
                src = bias_d.ap()
                nc.scalar.dma_start(
                    t[:, :],
                    bass.AP(tensor=src.tensor, offset=src.offset,
                            ap=[[1, ST], [ST, NG]]))
                return t

            def load_w_pair(pool, w_d, tag):
                # [128, NDC, NG, 128]: stationary block for K^T/Q^T projection
                t = pool.tile([ST, NDC, NG, ST], BF16, name=tag, tag=tag, bufs=1)
                for h in range(H):
                    src = w_d.ap()[h].rearrange("(c p) e -> p c e", p=ST)
                    wstg = pool.tile([ST, NDC, HD], F32, tag="wstg")
                    nc.scalar.dma_start(wstg[:, :, :], src)
                    nc.vector.tensor_copy(
                        t[:, :, h // 2, (h % 2) * HD:(h % 2) * HD + HD],
                        wstg[:, :, :])
                return t

            def load_w_flat(pool, w_d, tag):
                # [128, NDC, H, HD]: moving operand for V projection
                t = pool.tile([ST, NDC, H, HD], BF16, name=tag, tag=tag, bufs=1)
                for h in range(H):
                    src = w_d.ap()[h].rearrange("(c p) e -> p c e", p=ST)
                    wstg = pool.tile([ST, NDC, HD], F32, tag="wstg")
                    nc.scalar.dma_start(wstg[:, :, :], src)
                    nc.vector.tensor_copy(t[:, :, h, :], wstg[:, :, :])
                return t

            # ---- phase 1: x^T, V, K^T, Q^T -----------------------------
            with (
                tc.tile_pool(name="p1a", bufs=2) as p1a,
                tc.tile_pool(name="pw", bufs=1) as pw,
            ):
                wkp = load_w_pair(pw, wk_d, "wkp")
                bkp = load_bias_pair(pw, bk_d, "bkp")
                bqp = load_bias_pair(pw, bq_d, "bqp")
                bvf = pw.tile([ST, H, HD], F32, name="bvf", tag="bvf", bufs=1)
                srcv = bv_d.ap()
                nc.sync.dma_start(
                    bvf[:, :, :],
                    bass.AP(tensor=srcv.tensor, offset=srcv.offset,
                            ap=[[0, ST]] + list(srcv.ap)))
                xt = {}
                with (
                    tc.tile_pool(name="xtp", bufs=1, side="right") as xtp,
                ):
                  for c in range(NDC):
                    for sg in range(NSG):
                        xt[c, sg] = xtp.tile([ST, 512], BF16,
                                             name=f"xt{c}_{sg}", tag=f"xt{c}_{sg}")
                  with (
                    tc.tile_pool(name="pv", bufs=1) as pv,
                    tc.tile_pool(name="ppsv", bufs=2, space="PSUM") as ppsa,
                    tc.tile_pool(name="ppst", bufs=3, space="PSUM") as ppst,
                  ):
                    wvb = load_w_flat(pv, wv_d, "wvb")
                    for sg in range(NSG):
                        for st in range(4 * sg, 4 * sg + 4):
                            so = (st % 4) * ST
                            xf = p1a.tile([ST, D], F32, tag="xf")
                            nc.sync.dma_start(
                                xf[:, :], x_d.ap()[st * ST:(st + 1) * ST, :])
                            xb = p1a.tile([ST, D], BF16, tag="xb")
                            nc.vector.tensor_copy(xb[:, :], xf[:, :])
                            for c in range(NDC):
                                pst = ppst.tile([ST, ST], BF16, tag="pst")
                                nc.tensor.transpose(
                                    pst[:, :], xb[:, c * ST:(c + 1) * ST],
                                    ident[:, :])
                                nc.scalar.copy(xt[c, sg][:, so:so + ST],
                                               pst[:, :])
                        for st in range(4 * sg, 4 * sg + 4):
                            so = (st % 4) * ST
                            psv = ppsa.tile([ST, H * HD], F32, tag="psv")
                            for c in range(NDC):
                                for n in range(2):
                                    nc.tensor.matmul(
                                        psv[:, n * 512:(n + 1) * 512],
                                        xt[c, sg][:, so:so + ST],
                                        wvb[:, c, 8 * n:8 * n + 8, :],
                                        start=(c == 0), stop=(c == NDC - 1))
                            nc.vector.tensor_add(
                                vb[:, st, :, 0:HD],
                                psv[:, :].rearrange("p (h e) -> p h e", h=H),
                                bvf[:, :, :])

                # Q^T: from the host-fed local query rows (xq), via the
                # same PE-transpose path, weight-pair stationary.
                NQG = NJ // 4
                with (
                    tc.tile_pool(name="pq", bufs=1) as pq,
                    tc.tile_pool(name="xqtp", bufs=1) as xqtp,
                    tc.tile_pool(name="ppsq", bufs=2, space="PSUM") as ppsq,
                    tc.tile_pool(name="ppstq", bufs=3, space="PSUM") as ppstq,
                ):
                    wqp = load_w_pair(pq, wq_d, "wqp")
                    xqt = {}
                    for c in range(NDC):
                        for sg in range(NQG):
                            xqt[c, sg] = xqtp.tile([ST, 512], BF16,
                                                   name=f"xqt{c}_{sg}",
                                                   tag=f"xqt{c}_{sg}")
                    for jl in range(NJ):
                        sg, so = jl // 4, (jl % 4) * ST
                        xf = p1a.tile([ST, D], F32, tag="xf")
                        nc.sync.dma_start(xf[:, :],
                                          xq_d.ap()[jl * ST:(jl + 1) * ST, :])
                        xb = p1a.tile([ST, D], BF16, tag="xb")
                        nc.vector.tensor_copy(xb[:, :], xf[:, :])
                        for c in range(NDC):
                            pst = ppstq.tile([ST, ST], BF16, tag="pstq")
                            nc.tensor.transpose(
                                pst[:, :], xb[:, c * ST:(c + 1) * ST], ident[:, :])
                            nc.vector.tensor_copy(xqt[c, sg][:, so:so + ST],
                                                  pst[:, :])
                    for g in range(NG):
                        psq = [ppsq.tile([ST, 512], F32, name=f"psq{sg}",
                                         tag=f"psq{sg}") for sg in range(NQG)]
                        for c in range(NDC):
                            for sg in range(NQG):
                                nc.tensor.matmul(
                                    psq[sg][:, :],
                                    wqp[:, c, g, :],
                                    xqt[c, sg][:, :],
                                    start=(c == 0), stop=(c == NDC - 1))
                        for sg in range(NQG):
                            nc.vector.tensor_scalar(
                                qt_t[g][:, sg * 512:(sg + 1) * 512],
                                psq[sg][:, :], bqp[:, g:g + 1], None,
                                mybir.AluOpType.add)

                # K^T: weight-pair stationary, x^T moving
                with (
                    tc.tile_pool(name="ppsk", bufs=2, space="PSUM") as ppsk,
                ):
                    for g in range(NG):
                        psk = [ppsk.tile([ST, 512], F32, name=f"psk{sg}",
                                         tag=f"psk{sg}") for sg in range(NSG)]
                        for c in range(NDC):
                            for sg in range(NSG):
                                nc.tensor.matmul(
                                    psk[sg][:, :],
                                    wkp[:, c, g, :],
                                    xt[c, sg][:, :],
                                    start=(c == 0), stop=(c == NDC - 1))
                        for sg in range(NSG):
                            nc.vector.tensor_scalar(
                                kt_t[g][:, sg * 512:(sg + 1) * 512],
                                psk[sg][:, :], bkp[:, g:g + 1], None,
                                mybir.AluOpType.add)

            # ---- phase 2: attention ------------------------------------
            late_cm = tc.tile_pool(name="late", bufs=1)
            late = late_cm.__enter__()
            cat = [late.tile([ST, NJ * ST], BF16, name=f"catg{g}",
                             tag=f"catg{g}") for g in range(NG)]
            mtb = late.tile([ST, max(n_maskt, 1), ST], BF16, name="mtb",
                            tag="mtb")
            wob = late.tile([ST, NDC, D], BF16, name="wob", tag="wob")
            nc.sync.dma_start(mtb[:, :, :], mt_d.ap().rearrange("m p f -> p m f"))
            with (
                tc.tile_pool(name="p2s", bufs=4) as p2s,
                tc.tile_pool(name="ldp", bufs=1, space="DRAM") as ldp,
                tc.tile_pool(name="pss", bufs=4, space="PSUM") as pss,
                tc.tile_pool(name="pso", bufs=2, space="PSUM") as pso,
            ):
                for c in range(NDC):
                    wstg = p2s.tile([ST, D], F32, tag="wstg3", bufs=2)
                    nc.sync.dma_start(wstg[:, :],
                                      wo_d.ap()[c * ST:(c + 1) * ST, :])
                    nc.vector.tensor_copy(wob[:, c, :], wstg[:, :])
                ldram = ldp.tile([H, NJ * ST], F32, name="ldram", tag="ld")
                for h in range(H):
                    g, r = h // 2, (h % 2) * HD
                    po = pso.tile([HD + 1, NJ * ST], F32, tag="po")
                    for b_ in range(NB):
                        if bank_slots[b_] and not bank_fast[b_]:
                            nc.vector.memset(
                                po[:, b_ * 512:(b_ + 1) * 512], 0.0)

                    def emit_av(k, runs, pt):
                        for run in runs:
                            sub = [run[0]]
                            subs = []
                            for j in run[1:]:
                                if kfirst[j] == kfirst[sub[0]]:
                                    sub.append(j)
                                else:
                                    subs.append(sub)
                                    sub = [j]
                            subs.append(sub)
                            for sub_ in subs:
                                ja, jb = sub_[0], sub_[-1]
                                b_ = ja // 4
                                fast = bank_fast[b_]
                                nc.tensor.matmul(
                                    po[0:HD + 1, ja * ST:(jb + 1) * ST],
                                    vb[:, k, h, :],
                                    pt[:, ja * ST:(jb + 1) * ST],
                                    start=(fast and k == kfirst[ja]),
                                    stop=(fast and k == bklast[b_]),
                                    skip_group_check=not fast)

                    pending = []
                    for k in range(NKT):
                        sl = slots_k[k]
                        if not sl:
                            continue
                        runs = []
                        run = [sl[0]]
                        for j in sl[1:]:
                            if j == run[-1] + 1 and j // 4 == run[0] // 4:
                                run.append(j)
                            else:
                                runs.append(run)
                                run = [j]
                        runs.append(run)
                        pt = p2s.tile([ST, NJ * ST], BF16, tag="pt", bufs=6)
                        for run in runs:
                            ja, jb = run[0], run[-1]
                            w_ = (jb + 1 - ja) * ST
                            psc = pss.tile([ST, 512], F32, tag="psc")
                            nc.tensor.matmul(
                                psc[:, 0:w_],
                                kt_t[g][r:r + HD, k * ST:(k + 1) * ST],
                                qt_t[g][r:r + HD, ja * ST:(jb + 1) * ST],
                                start=True, stop=True)
                            nc.scalar.activation(
                                pt[:, ja * ST:(jb + 1) * ST], psc[:, 0:w_],
                                mybir.ActivationFunctionType.Exp,
                                scale=1.0 / float(np.sqrt(HD)))
                        for j in sl:
                            if cls[j, k] == 2:
                                m = mixed_idx[(j, k)]
                                nc.vector.tensor_mul(
                                    pt[:, j * ST:(j + 1) * ST],
                                    pt[:, j * ST:(j + 1) * ST],
                                    mtb[:, m, :])
                        pending.append((k, runs, pt))
                        if len(pending) > 2:
                            emit_av(*pending.pop(0))
                    for args in pending:
                        emit_av(*args)
                    # unnormalized head output; 1/l = exp(-ln(l)) on ScalarE
                    nc.vector.tensor_copy(cat[g][r:r + HD, :], po[0:HD, :])
                    ltmp = p2s.tile([1, NJ * ST], F32, tag="ltmp")
                    nc.vector.tensor_copy(ltmp[:, :], po[HD:HD + 1, :])
                    rec = p2s.tile([1, NJ * ST], F32, tag="rec")
                    nc.vector.reciprocal_approx_fast(rec[:, :], ltmp[:, :])
                    nc.sync.dma_start(ldram[h:h + 1, :], rec[:, :])
                    recb = p2s.tile([ST, NJ * ST], F32, tag="recb")
                    lsrc = ldram[h]
                    nc.sync.dma_start(
                        recb[r:r + HD, :],
                        bass.AP(tensor=lsrc.tensor, offset=lsrc.offset,
                                ap=[[0, HD]] + list(lsrc.ap)))
                    nc.vector.tensor_mul(cat[g][r:r + HD, :],
                                         cat[g][r:r + HD, :],
                                         recb[r:r + HD, :])

            # ---- phase 3: fc_out ---------------------------------------
            with (
                tc.tile_pool(name="p3s", bufs=3) as p3s,
                tc.tile_pool(name="psy", bufs=2, space="PSUM") as psy,
            ):
                for jt in range(NJ):
                    py = [psy.tile([ST, 512], F32, name=f"py{n}", tag=f"py{n}")
                          for n in range(2)]
                    for c in range(NDC):
                        for n in range(2):
                            nc.tensor.matmul(
                                py[n][:, :],
                                cat[c][:, jt * ST:(jt + 1) * ST],
                                wob[:, c, n * 512:(n + 1) * 512],
                                start=(c == 0), stop=(c == NDC - 1))
                    for n in range(2):
                        ysb = p3s.tile([ST, 512], F32, tag="ysb")
                        nc.vector.tensor_add(ysb[:, :], py[n][:, :],
                                             bob[:, n * 512:(n + 1) * 512])
                        nc.sync.dma_start(
                            out_d.ap()[jt * ST:(jt + 1) * ST,
                                       n * 512:(n + 1) * 512],
                            ysb[:, :])
            late_cm.__exit__(None, None, None)

    nc.compile()
    return nc


_CACHE = {}
LAST_RESULT = None


def _get_program(mask):
    key = mask.tobytes()
    if key not in _CACHE:
        cls, mixed = _classify(mask)
        _CACHE[key] = (_build(cls, mixed, max(len(mixed), 1)), cls, mixed)
    return _CACHE[key]


def kernel(x, mask, Wq, bq, Wk, bk, Wv, bv, Wo, bo):
    x = np.ascontiguousarray(np.asarray(x, dtype=np.float32))
    mask = np.asarray(mask)
    nc, cls, mixed = _get_program(mask)

    n_maskt = max(len(mixed), 1)
    base = {
        "wq": np.ascontiguousarray(Wq, dtype=np.float32),
        "wk": np.ascontiguousarray(Wk, dtype=np.float32),
        "wv": np.ascontiguousarray(Wv, dtype=np.float32),
        "wo": np.ascontiguousarray(Wo, dtype=np.float32),
        "bq": np.ascontiguousarray(bq, dtype=np.float32),
        "bk": np.ascontiguousarray(bk, dtype=np.float32),
        "bv": np.ascontiguousarray(bv, dtype=np.float32),
        "bo": np.ascontiguousarray(bo, dtype=np.float32),
    }
    in_maps = []
    for c in range(N_CORES):
        b, h = c // 2, c % 2
        qrows = np.concatenate(
            [np.arange((2 * j + h) * ST, (2 * j + h + 1) * ST) for j in range(NJ)])
        mt = np.zeros((n_maskt, ST, ST), dtype=ml_dtypes.bfloat16)
        for i, (j, k) in enumerate(mixed):
            blk = mask[(2 * j + h) * ST:(2 * j + h + 1) * ST,
                       k * ST:(k + 1) * ST]
            mt[i] = (blk != 0).T.astype(ml_dtypes.bfloat16)
        m = dict(base)
        m["x"] = x[b]
        m["xq"] = np.ascontiguousarray(x[b][qrows])
        m["maskt"] = mt
        in_maps.append(m)

    res = run_bass_kernel_spmd(
        nc, in_maps, core_ids=list(range(N_CORES)),
        trace=os.environ.get("BASS_KERNEL_TRACE", "0") == "1")
    global LAST_RESULT
    LAST_RESULT = res

    out = np.empty((B, S, D), dtype=np.float32)
    for c in range(N_CORES):
        b, h = c // 2, c % 2
        oc = res.results[c]["out"]
        for j in range(NJ):
            out[b, (2 * j + h) * ST:(2 * j + h + 1) * ST, :] = \
                oc[j * ST:(j + 1) * ST, :]
    return out



# revision 5
# speedup vs baseline: 1.5645x; 1.5645x over previous
"""Trainium2 Bass kernel: causal multi-head attention (B=4,S=2048,D=1024,H=16).

Sharding (8 cores, host-side pair reduction): core c -> batch b=c//2,
head-half hh=c%2 (local heads hh*8..hh*8+7, i.e. 4 head pairs).  Each core
computes Q/K/V for its 8 heads over ALL 2048 rows, full causal attention,
and a PARTIAL fc_out against the row-shard Wo[hh*512:(hh+1)*512].  The host
sums the two partials per batch (the "all-reduce" of the row-sharded Wo).

Device pipeline per core (all matmuls bf16, f32 accumulation):
  A: x tiles DMA (bf16, host-cast) -> PE transposes -> x^T; V = x@Wv (8 heads
     wide, N=512); K^T/Q^T for pair 0.
  B: per pair g, per q-chunk of 512 cols, per k-tile:
     scores^T pair = two row-tiled concurrent matmuls (heads at array rows
     0-63 / 64-127) -> one 1024-wide exp on ScalarE (scale=1/8 folded,
     PSUM->SBUF bf16), 0/1 mask multiply on diag blocks, ones-augmented AV
     accumulation (row 64 = softmax denominator).  K^T/Q^T of pair g+1 are
     emitted interleaved so the PE fills exp-wait gaps.
     Finalize: reciprocal of denominators (DVE), K=1 broadcast matmul,
     normalize into cat (bf16).
  C: partial fc_out = cat.T @ Wo_shard (+bo on hh==0 cores only), bf16 out.

Weights are pre-packed on the host into the exact stationary layouts (bf16),
so there is no device-side weight rearrangement.  The program is specialized
at build time to the mask's 128x128 block structure (computed from the actual
mask input, so it stays correct for any mask).
"""

import os
import numpy as np
import ml_dtypes

import concourse.bass as bass
import concourse.mybir as mybir
import concourse.tile as tile
from concourse import bacc
from concourse.bass_utils import run_bass_kernel_spmd
from concourse.masks import make_identity

B, S, D, H, HD = 4, 2048, 1024, 16, 64
N_CORES = 8
ST = 128                 # tile edge
NKT = S // ST            # 16 k tiles
NQT = S // ST            # 16 q tiles
NDC = D // ST            # 8 contraction chunks
HL = H // 2              # 8 local heads per core
NG = HL // 2             # 4 local head pairs
NQC = 4                  # q chunks per core
QCW = S // NQC           # 512 cols per q chunk (4 q tiles)
QCT = QCW // ST          # 4 q tiles per chunk

F32 = mybir.dt.float32
BF16 = mybir.dt.bfloat16
BF = ml_dtypes.bfloat16


def _classify(mask: np.ndarray):
    """128x128 block structure of the mask: 0 skip, 1 full, 2 mixed."""
    cls = np.zeros((NQT, NKT), dtype=int)
    for j in range(NQT):
        for k in range(NKT):
            blk = mask[j * ST:(j + 1) * ST, k * ST:(k + 1) * ST]
            if (blk != 0).all():
                cls[j, k] = 1
            elif (blk == 0).all():
                cls[j, k] = 0
            else:
                cls[j, k] = 2
    mixed = [(j, k) for j in range(NQT) for k in range(NKT) if cls[j, k] == 2]
    return cls, mixed


def _runs(valid):
    """Contiguous runs [(ja, jb)] of a sorted list of chunk-local j."""
    runs = []
    for j in valid:
        if runs and j == runs[-1][1] + 1:
            runs[-1][1] = j
        else:
            runs.append([j, j])
    return [(a, b) for a, b in runs]


def _build(cls, mixed, n_maskt):
    nc = bacc.Bacc("TRN2", target_bir_lowering=False, debug=False,
                   num_devices=N_CORES)

    x_d = nc.dram_tensor("x", [S, D], BF16, kind="ExternalInput")
    wqp_d = nc.dram_tensor("wqp", [ST, NDC, NG, ST], BF16, kind="ExternalInput")
    wkp_d = nc.dram_tensor("wkp", [ST, NDC, NG, ST], BF16, kind="ExternalInput")
    wvb_d = nc.dram_tensor("wvb", [ST, NDC, HL * HD], BF16, kind="ExternalInput")
    wob_d = nc.dram_tensor("wob", [ST, NG, D], BF16, kind="ExternalInput")
    bqp_d = nc.dram_tensor("bqp", [ST, NG], F32, kind="ExternalInput")
    bkp_d = nc.dram_tensor("bkp", [ST, NG], F32, kind="ExternalInput")
    bvf_d = nc.dram_tensor("bvf", [HL, HD], F32, kind="ExternalInput")
    bob_d = nc.dram_tensor("bob", [D], F32, kind="ExternalInput")
    mt_d = nc.dram_tensor("maskt", [n_maskt, ST, ST], BF16, kind="ExternalInput")
    out_d = nc.dram_tensor("out", [S, D], BF16, kind="ExternalOutput")

    mixed_idx = {jk: i for i, jk in enumerate(mixed)}

    # per (qc): k list + per-k valid chunk-local j's
    chunk_ks = []
    chunk_vj = []
    for qc in range(NQC):
        vj = {}
        for k in range(NKT):
            v = [j for j in range(QCT) if cls[qc * QCT + j, k]]
            if v:
                vj[k] = v
        ks = sorted(vj)
        chunk_ks.append(ks)
        chunk_vj.append(vj)

    with tile.TileContext(nc) as tc:
        with tc.tile_pool(name="pp", bufs=1) as pp:
            # ---- persistent SBUF ----------------------------------------
            kt = [pp.tile([ST, S], BF16, name=f"kt{g}", tag=f"kt{g}")
                  for g in range(NG)]
            qt = [pp.tile([ST, S], BF16, name=f"qt{g}", tag=f"qt{g}")
                  for g in range(NG)]
            cat = [pp.tile([ST, S], BF16, name=f"cat{g}", tag=f"cat{g}")
                   for g in range(NG)]
            xt = [pp.tile([ST, S], BF16, name=f"xt{c}", tag=f"xt{c}")
                  for c in range(NDC)]
            vb = pp.tile([ST, NKT, HL, HD + 1], BF16, name="vb", tag="vb")
            wqp = pp.tile([ST, NDC, NG, ST], BF16, name="wqp", tag="wqp")
            wkp = pp.tile([ST, NDC, NG, ST], BF16, name="wkp", tag="wkp")
            wvb = pp.tile([ST, NDC, HL * HD], BF16, name="wvb", tag="wvb")
            wob = pp.tile([ST, NG, D], BF16, name="wob", tag="wob")
            bqp = pp.tile([ST, NG], F32, name="bqp", tag="bqp")
            bkp = pp.tile([ST, NG], F32, name="bkp", tag="bkp")
            bvf = pp.tile([ST, HL, HD], F32, name="bvf", tag="bvf")
            bob = pp.tile([ST, D], F32, name="bob", tag="bob")
            mtb = pp.tile([ST, max(n_maskt, 1), ST], BF16, name="mtb", tag="mtb")
            ident = pp.tile([ST, ST], BF16, name="ident", tag="ident")
            ones64 = pp.tile([1, HD], F32, name="ones64", tag="ones64")

            # weight/bias DMAs spread across queues; x on sync (emitted in
            # epoch A below so the first tiles land first on that queue).
            nc.scalar.dma_start(wvb[:, :, :], wvb_d.ap())
            nc.scalar.dma_start(bqp[:, :], bqp_d.ap())
            nc.scalar.dma_start(bkp[:, :], bkp_d.ap())
            src = bvf_d.ap()
            nc.scalar.dma_start(
                bvf[:, :, :],
                bass.AP(tensor=src.tensor, offset=src.offset,
                        ap=[[0, ST]] + list(src.ap)))
            nc.gpsimd.dma_start(wkp[:, :, :, :], wkp_d.ap())
            nc.gpsimd.dma_start(wqp[:, :, :, :], wqp_d.ap())
            nc.gpsimd.dma_start(wob[:, :, :], wob_d.ap())
            nc.gpsimd.dma_start(mtb[:, :, :],
                                mt_d.ap().rearrange("m p f -> p m f"))
            src = bob_d.ap()
            nc.gpsimd.dma_start(
                bob[:, :],
                bass.AP(tensor=src.tensor, offset=src.offset,
                        ap=[[0, ST]] + list(src.ap)))

            nc.vector.memset(vb[:, :, :, HD:HD + 1], 1.0)
            nc.vector.memset(ones64[:, :], 1.0)
            make_identity(nc, ident[:, :])

            def emit_proj(g, ppaux):
                """K^T/Q^T projection for pair g; yields for interleaving."""
                for w_t, bias_t, dst in ((wkp, bkp, kt[g]), (wqp, bqp, qt[g])):
                    for sg in range(4):
                        ps = ppaux.tile([ST, 512], F32, tag="pskq")
                        for c in range(NDC):
                            nc.tensor.matmul(
                                ps[:, :], w_t[:, c, g, :],
                                xt[c][:, sg * 512:(sg + 1) * 512],
                                start=(c == 0), stop=(c == NDC - 1),
                                skip_group_check=True)
                            if c % 4 == 3:
                                yield
                        nc.vector.tensor_scalar(
                            dst[:, sg * 512:(sg + 1) * 512], ps[:, :],
                            bias_t[:, g:g + 1], None, mybir.AluOpType.add)
                        yield

            # ---- epoch A: x^T, V, K0/Q0 ---------------------------------
            with (
                tc.tile_pool(name="pxb", bufs=3) as pxb,
                tc.tile_pool(name="ppst", bufs=3, space="PSUM") as ppst,
                tc.tile_pool(name="ppsv", bufs=2, space="PSUM") as ppsv,
                tc.tile_pool(name="ppkq0", bufs=2, space="PSUM") as ppkq0,
            ):
                for st in range(NKT):
                    xb = pxb.tile([ST, D], BF16, tag="xb")
                    nc.sync.dma_start(xb[:, :], x_d.ap()[st * ST:(st + 1) * ST, :])
                    for c in range(NDC):
                        pst = ppst.tile([ST, ST], BF16, tag="pst")
                        nc.tensor.transpose(
                            pst[:, :], xb[:, c * ST:(c + 1) * ST], ident[:, :])
                        nc.scalar.copy(xt[c][:, st * ST:(st + 1) * ST], pst[:, :])
                    psv = ppsv.tile([ST, HL * HD], F32, tag="psv")
                    for c in range(NDC):
                        nc.tensor.matmul(
                            psv[:, :], xt[c][:, st * ST:(st + 1) * ST],
                            wvb[:, c, :], start=(c == 0), stop=(c == NDC - 1))
                    nc.vector.tensor_add(
                        vb[:, st, :, 0:HD],
                        psv[:, :].rearrange("p (h e) -> p h e", h=HL),
                        bvf[:, :, :])
                for _ in emit_proj(0, ppkq0):
                    pass

            # ---- epoch B: attention + interleaved projections -----------
            with (
                tc.tile_pool(name="pscores", bufs=2, space="PSUM") as pscores,
                tc.tile_pool(name="ppo", bufs=1, space="PSUM") as ppo,
                tc.tile_pool(name="ppaux", bufs=1, space="PSUM") as ppaux,
                tc.tile_pool(name="ppt", bufs=3) as ppt,
                tc.tile_pool(name="pfin", bufs=2) as pfin,
            ):
                for g in range(NG):
                    em = emit_proj(g + 1, ppaux) if g + 1 < NG else iter(())
                    for qc in range(NQC):
                        ks = chunk_ks[qc]
                        vjm = chunk_vj[qc]
                        if not ks:
                            continue
                        union = sorted({j for v in vjm.values() for j in v})
                        fast = vjm[ks[0]] == union
                        po = ppo.tile([HD + 1, 2 * QCW], F32, tag="po")
                        if not fast:
                            nc.vector.memset(po[:, :], 0.0)
                        for idx, k in enumerate(ks):
                            runs = _runs(vjm[k])
                            psc = pscores.tile([ST, 2 * QCW], F32, tag="psc")
                            for ja, jb in runs:
                                for h in range(2):
                                    nc.tensor.matmul(
                                        psc[:, h * QCW + ja * ST:
                                            h * QCW + (jb + 1) * ST],
                                        kt[g][h * HD:(h + 1) * HD,
                                              k * ST:(k + 1) * ST],
                                        qt[g][h * HD:(h + 1) * HD,
                                              qc * QCW + ja * ST:
                                              qc * QCW + (jb + 1) * ST],
                                        start=True, stop=True)
                            pt = ppt.tile([ST, 2 * QCW], BF16, tag="pt")
                            nc.scalar.activation(
                                pt[:, :], psc[:, :],
                                mybir.ActivationFunctionType.Exp,
                                scale=1.0 / float(np.sqrt(HD)))
                            for j in vjm[k]:
                                if cls[qc * QCT + j, k] == 2:
                                    m = mixed_idx[(qc * QCT + j, k)]
                                    for h in range(2):
                                        nc.vector.tensor_mul(
                                            pt[:, h * QCW + j * ST:
                                               h * QCW + (j + 1) * ST],
                                            pt[:, h * QCW + j * ST:
                                               h * QCW + (j + 1) * ST],
                                            mtb[:, m, :])
                            for h in range(2):
                                for ja, jb in runs:
                                    nc.tensor.matmul(
                                        po[0:HD + 1,
                                           h * QCW + ja * ST:
                                           h * QCW + (jb + 1) * ST],
                                        vb[:, k, 2 * g + h, :],
                                        pt[:, h * QCW + ja * ST:
                                           h * QCW + (jb + 1) * ST],
                                        start=(fast and idx == 0),
                                        stop=(fast and idx == len(ks) - 1),
                                        skip_group_check=True)
                            next(em, None)
                            next(em, None)
                        # finalize (g, qc)
                        ltmp = pfin.tile([1, 2 * QCW], F32, tag="ltmp")
                        nc.vector.tensor_copy(ltmp[:, :], po[HD:HD + 1, :])
                        rec = pfin.tile([1, 2 * QCW], F32, tag="rec")
                        nc.vector.reciprocal_approx_fast(rec[:, :], ltmp[:, :])
                        for h in range(2):
                            rb = ppaux.tile([HD, QCW], F32, tag="rb")
                            nc.tensor.matmul(
                                rb[:, :], ones64[0:1, :],
                                rec[0:1, h * QCW:(h + 1) * QCW],
                                start=True, stop=True)
                            rbs = pfin.tile([HD, QCW], F32, tag="rbs")
                            nc.vector.tensor_copy(rbs[:, :], rb[:, :])
                            nc.vector.tensor_mul(
                                cat[g][h * HD:(h + 1) * HD,
                                       qc * QCW:(qc + 1) * QCW],
                                po[0:HD, h * QCW:(h + 1) * QCW],
                                rbs[:, :])
                    for _ in em:
                        pass

            # ---- epoch C: partial fc_out --------------------------------
            with (
                tc.tile_pool(name="pfc", bufs=2, space="PSUM") as pfc,
                tc.tile_pool(name="pfcs", bufs=3) as pfcs,
            ):
                for jt in range(NQT):
                    py = pfc.tile([ST, D], F32, tag="py")
                    for g in range(NG):
                        for n in range(2):
                            nc.tensor.matmul(
                                py[:, n * 512:(n + 1) * 512],
                                cat[g][:, jt * ST:(jt + 1) * ST],
                                wob[:, g, n * 512:(n + 1) * 512],
                                start=(g == 0), stop=(g == NG - 1))
                    ysb = pfcs.tile([ST, D], BF16, tag="ysb")
                    nc.vector.tensor_add(ysb[:, :], py[:, :], bob[:, :])
                    nc.sync.dma_start(
                        out_d.ap()[jt * ST:(jt + 1) * ST, :], ysb[:, :])

    nc.compile()
    return nc


_CACHE = {}
LAST_RESULT = None


def _get_program(mask):
    key = mask.tobytes()
    if key not in _CACHE:
        cls, mixed = _classify(mask)
        _CACHE[key] = (_build(cls, mixed, max(len(mixed), 1)), cls, mixed)
    return _CACHE[key]


def kernel(x, mask, Wq, bq, Wk, bk, Wv, bv, Wo, bo):
    x = np.asarray(x, dtype=np.float32)
    mask = np.asarray(mask)
    Wq = np.asarray(Wq, dtype=np.float32)
    Wk = np.asarray(Wk, dtype=np.float32)
    Wv = np.asarray(Wv, dtype=np.float32)
    Wo = np.asarray(Wo, dtype=np.float32)
    nc, cls, mixed = _get_program(mask)

    n_maskt = max(len(mixed), 1)
    mt = np.zeros((n_maskt, ST, ST), dtype=BF)
    for i, (j, k) in enumerate(mixed):
        blk = mask[j * ST:(j + 1) * ST, k * ST:(k + 1) * ST]
        mt[i] = (blk != 0).T.astype(BF)

    def pack_pair(W, hh):
        # [128, NDC, NG, 128]: [p, c, g, m*64+e] = W[8hh + 2g+m, 128c+p, e]
        Wl = W[hh * HL:(hh + 1) * HL].reshape(NG, 2, NDC, ST, HD)
        return np.ascontiguousarray(
            Wl.transpose(3, 2, 0, 1, 4).reshape(ST, NDC, NG, ST).astype(BF))

    in_maps = []
    for c in range(N_CORES):
        b, hh = c // 2, c % 2
        Wvl = Wv[hh * HL:(hh + 1) * HL].reshape(HL, NDC, ST, HD)
        wvb = np.ascontiguousarray(
            Wvl.transpose(2, 1, 0, 3).reshape(ST, NDC, HL * HD).astype(BF))
        Wol = Wo[hh * HL * HD:(hh + 1) * HL * HD].reshape(NG, 2, HD, D)
        wob = np.ascontiguousarray(
            Wol.transpose(1, 2, 0, 3).reshape(ST, NG, D).astype(BF))
        bql = np.asarray(bq, dtype=np.float32)[hh * HL:(hh + 1) * HL]
        bkl = np.asarray(bk, dtype=np.float32)[hh * HL:(hh + 1) * HL]
        bqp = np.ascontiguousarray(
            bql.reshape(NG, 2, HD).transpose(1, 2, 0).reshape(ST, NG))
        bkp = np.ascontiguousarray(
            bkl.reshape(NG, 2, HD).transpose(1, 2, 0).reshape(ST, NG))
        m = {
            "x": np.ascontiguousarray(x[b].astype(BF)),
            "wqp": pack_pair(Wq, hh),
            "wkp": pack_pair(Wk, hh),
            "wvb": wvb,
            "wob": wob,
            "bqp": bqp,
            "bkp": bkp,
            "bvf": np.ascontiguousarray(
                np.asarray(bv, dtype=np.float32)[hh * HL:(hh + 1) * HL]),
            "bob": (np.asarray(bo, dtype=np.float32) if hh == 0
                    else np.zeros(D, dtype=np.float32)),
            "maskt": mt,
        }
        in_maps.append(m)

    res = run_bass_kernel_spmd(
        nc, in_maps, core_ids=list(range(N_CORES)),
        trace=os.environ.get("BASS_KERNEL_TRACE", "0") == "1")
    global LAST_RESULT
    LAST_RESULT = res

    out = np.empty((B, S, D), dtype=np.float32)
    for b in range(B):
        out[b] = (res.results[2 * b]["out"].astype(np.float32)
                  + res.results[2 * b + 1]["out"].astype(np.float32))
    return out


# revision 10
# speedup vs baseline: 1.5870x; 1.0144x over previous
"""Trainium2 Bass kernel: causal multi-head attention (B=4,S=2048,D=1024,H=16).

Sharding (8 cores, host-side pair reduction): core c -> batch b=c//2,
head-half hh=c%2 (local heads hh*8..hh*8+7, i.e. 4 head pairs).  Each core
computes Q/K/V for its 8 heads over ALL 2048 rows, full causal attention,
and a PARTIAL fc_out against the row-shard Wo[hh*512:(hh+1)*512].  The host
sums the two partials per batch (the "all-reduce" of the row-sharded Wo).

Device pipeline per core (all matmuls bf16, f32 accumulation):
  A: x tiles DMA (bf16, host-cast) -> PE transposes -> x^T; V = x@Wv (8 heads
     wide, N=512); K^T/Q^T for pair 0.
  B: per pair g, per q-chunk of 512 cols, per k-tile:
     scores^T pair = two row-tiled concurrent matmuls (heads at array rows
     0-63 / 64-127) -> one 1024-wide exp on ScalarE (scale=1/8 folded,
     PSUM->SBUF bf16), 0/1 mask multiply on diag blocks, ones-augmented AV
     accumulation (row 64 = softmax denominator).  K^T/Q^T of pair g+1 are
     emitted interleaved so the PE fills exp-wait gaps.
     Finalize: reciprocal of denominators (DVE), K=1 broadcast matmul,
     normalize into cat (bf16).
  C: partial fc_out = cat.T @ Wo_shard (+bo on hh==0 cores only), bf16 out.

Weights are pre-packed on the host into the exact stationary layouts (bf16),
so there is no device-side weight rearrangement.  The program is specialized
at build time to the mask's 128x128 block structure (computed from the actual
mask input, so it stays correct for any mask).
"""

import os
import numpy as np
import ml_dtypes

import concourse.bass as bass
import concourse.mybir as mybir
import concourse.tile as tile
from concourse import bacc
from concourse.bass_utils import run_bass_kernel_spmd
from concourse.masks import make_identity

B, S, D, H, HD = 4, 2048, 1024, 16, 64
N_CORES = 8
ST = 128                 # tile edge
NKT = S // ST            # 16 k tiles
NQT = S // ST            # 16 q tiles
NDC = D // ST            # 8 contraction chunks
HL = H // 2              # 8 local heads per core
NG = HL // 2             # 4 local head pairs
NQC = 4                  # q chunks per core
QCW = S // NQC           # 512 cols per q chunk (4 q tiles)
QCT = QCW // ST          # 4 q tiles per chunk

F32 = mybir.dt.float32
BF16 = mybir.dt.bfloat16
BF = ml_dtypes.bfloat16


def _classify(mask: np.ndarray):
    """128x128 block structure of the mask: 0 skip, 1 full, 2 mixed."""
    cls = np.zeros((NQT, NKT), dtype=int)
    for j in range(NQT):
        for k in range(NKT):
            blk = mask[j * ST:(j + 1) * ST, k * ST:(k + 1) * ST]
            if (blk != 0).all():
                cls[j, k] = 1
            elif (blk == 0).all():
                cls[j, k] = 0
            else:
                cls[j, k] = 2
    mixed = [(j, k) for j in range(NQT) for k in range(NKT) if cls[j, k] == 2]
    return cls, mixed


def _runs(valid):
    """Contiguous runs [(ja, jb)] of a sorted list of chunk-local j."""
    runs = []
    for j in valid:
        if runs and j == runs[-1][1] + 1:
            runs[-1][1] = j
        else:
            runs.append([j, j])
    return [(a, b) for a, b in runs]


def _build(cls, mixed, n_maskt):
    nc = bacc.Bacc("TRN2", target_bir_lowering=False, debug=False,
                   num_devices=N_CORES)

    x_d = nc.dram_tensor("x", [S, D], BF16, kind="ExternalInput")
    wqp_d = nc.dram_tensor("wqp", [ST, NDC, NG, ST], BF16, kind="ExternalInput")
    wkp_d = nc.dram_tensor("wkp", [ST, NDC, NG, ST], BF16, kind="ExternalInput")
    wvb_d = nc.dram_tensor("wvb", [ST, NDC, HL * HD], BF16, kind="ExternalInput")
    wob_d = nc.dram_tensor("wob", [ST, NG, D], BF16, kind="ExternalInput")
    bqp_d = nc.dram_tensor("bqp", [ST, NG], F32, kind="ExternalInput")
    bkp_d = nc.dram_tensor("bkp", [ST, NG], F32, kind="ExternalInput")
    bvf_d = nc.dram_tensor("bvf", [HL, HD], F32, kind="ExternalInput")
    bob_d = nc.dram_tensor("bob", [D], F32, kind="ExternalInput")
    mt_d = nc.dram_tensor("maskt", [n_maskt, ST, ST], BF16, kind="ExternalInput")
    out_d = nc.dram_tensor("out", [S, D], BF16, kind="ExternalOutput")

    mixed_idx = {jk: i for i, jk in enumerate(mixed)}

    # per (qc): k list + per-k valid chunk-local j's
    chunk_ks = []
    chunk_vj = []
    for qc in range(NQC):
        vj = {}
        for k in range(NKT):
            v = [j for j in range(QCT) if cls[qc * QCT + j, k]]
            if v:
                vj[k] = v
        ks = sorted(vj)
        chunk_ks.append(ks)
        chunk_vj.append(vj)

    with tile.TileContext(nc) as tc:
        with tc.tile_pool(name="pp", bufs=1) as pp:
            # ---- persistent SBUF ----------------------------------------
            kt = [pp.tile([ST, S], BF16, name=f"kt{g}", tag=f"kt{g}")
                  for g in range(NG)]
            qt = [pp.tile([ST, S], BF16, name=f"qt{g}", tag=f"qt{g}")
                  for g in range(NG)]
            cat = [pp.tile([ST, S], BF16, name=f"cat{g}", tag=f"cat{g}")
                   for g in range(NG)]
            xt = [pp.tile([ST, S], BF16, name=f"xt{c}", tag=f"xt{c}")
                  for c in range(NDC)]
            vb = pp.tile([ST, NKT, HL, HD + 1], BF16, name="vb", tag="vb")
            wqp = pp.tile([ST, NDC, NG, ST], BF16, name="wqp", tag="wqp")
            wkp = pp.tile([ST, NDC, NG, ST], BF16, name="wkp", tag="wkp")
            wvb = pp.tile([ST, NDC, HL * HD], BF16, name="wvb", tag="wvb")
            wob = pp.tile([ST, NG, D], BF16, name="wob", tag="wob")
            bqp = pp.tile([ST, NG], F32, name="bqp", tag="bqp")
            bkp = pp.tile([ST, NG], F32, name="bkp", tag="bkp")
            bvf = pp.tile([ST, HL, HD], F32, name="bvf", tag="bvf")
            bob = pp.tile([ST, D], F32, name="bob", tag="bob")
            mtb = pp.tile([ST, max(n_maskt, 1), ST], BF16, name="mtb", tag="mtb")
            ident = pp.tile([ST, ST], BF16, name="ident", tag="ident")

            # weight/bias DMAs spread across queues; x on sync (emitted in
            # epoch A below so the first tiles land first on that queue).
            nc.scalar.dma_start(wvb[:, :, :], wvb_d.ap())
            nc.scalar.dma_start(bqp[:, :], bqp_d.ap())
            nc.scalar.dma_start(bkp[:, :], bkp_d.ap())
            src = bvf_d.ap()
            nc.scalar.dma_start(
                bvf[:, :, :],
                bass.AP(tensor=src.tensor, offset=src.offset,
                        ap=[[0, ST]] + list(src.ap)))
            nc.gpsimd.dma_start(wkp[:, :, :, :], wkp_d.ap())
            nc.gpsimd.dma_start(wqp[:, :, :, :], wqp_d.ap())
            nc.gpsimd.dma_start(wob[:, :, :], wob_d.ap())
            nc.gpsimd.dma_start(mtb[:, :, :],
                                mt_d.ap().rearrange("m p f -> p m f"))
            src = bob_d.ap()
            nc.gpsimd.dma_start(
                bob[:, :],
                bass.AP(tensor=src.tensor, offset=src.offset,
                        ap=[[0, ST]] + list(src.ap)))

            nc.vector.memset(vb[:, :, :, HD:HD + 1], 1.0)
            make_identity(nc, ident[:, :])

            def emit_proj(g, ppaux):
                """K^T/Q^T projection for pair g; yields for interleaving."""
                for w_t, bias_t, dst in ((wkp, bkp, kt[g]), (wqp, bqp, qt[g])):
                    for sg in range(4):
                        ps = ppaux.tile([ST, 512], F32, tag="pskq")
                        for c in range(NDC):
                            nc.tensor.matmul(
                                ps[:, :], w_t[:, c, g, :],
                                xt[c][:, sg * 512:(sg + 1) * 512],
                                start=(c == 0), stop=(c == NDC - 1),
                                skip_group_check=True)
                            if c % 4 == 3:
                                yield
                        nc.vector.tensor_scalar(
                            dst[:, sg * 512:(sg + 1) * 512], ps[:, :],
                            bias_t[:, g:g + 1], None, mybir.AluOpType.add)
                        yield

            # ---- epoch A: x^T, V, K0/Q0 ---------------------------------
            with (
                tc.tile_pool(name="pxb", bufs=3) as pxb,
                tc.tile_pool(name="ppst", bufs=3, space="PSUM") as ppst,
                tc.tile_pool(name="ppsv", bufs=2, space="PSUM") as ppsv,
                tc.tile_pool(name="ppkq0", bufs=2, space="PSUM") as ppkq0,
            ):
                for st in range(NKT):
                    xb = pxb.tile([ST, D], BF16, tag="xb")
                    nc.sync.dma_start(xb[:, :], x_d.ap()[st * ST:(st + 1) * ST, :])
                    for c in range(NDC):
                        pst = ppst.tile([ST, ST], BF16, tag="pst")
                        nc.tensor.transpose(
                            pst[:, :], xb[:, c * ST:(c + 1) * ST], ident[:, :])
                        nc.vector.tensor_copy(
                            xt[c][:, st * ST:(st + 1) * ST], pst[:, :])
                    psv = ppsv.tile([ST, HL * HD], F32, tag="psv")
                    for c in range(NDC):
                        nc.tensor.matmul(
                            psv[:, :], xt[c][:, st * ST:(st + 1) * ST],
                            wvb[:, c, :], start=(c == 0), stop=(c == NDC - 1))
                    nc.vector.tensor_add(
                        vb[:, st, :, 0:HD],
                        psv[:, :].rearrange("p (h e) -> p h e", h=HL),
                        bvf[:, :, :])
                for _ in emit_proj(0, ppkq0):
                    pass

            # ---- epoch B: attention + interleaved projections -----------
            with (
                tc.tile_pool(name="pscores", bufs=2, space="PSUM") as pscores,
                tc.tile_pool(name="ppo", bufs=1, space="PSUM") as ppo,
                tc.tile_pool(name="ppaux", bufs=1, space="PSUM") as ppaux,
                tc.tile_pool(name="ppt", bufs=3) as ppt,
                tc.tile_pool(name="pfin", bufs=2) as pfin,
            ):
                for g in range(NG):
                    em = emit_proj(g + 1, ppaux) if g + 1 < NG else iter(())
                    for qc in range(NQC):
                        ks = chunk_ks[qc]
                        vjm = chunk_vj[qc]
                        if not ks:
                            continue
                        union = sorted({j for v in vjm.values() for j in v})
                        fast = vjm[ks[0]] == union
                        po = ppo.tile([HD + 1, 2 * QCW], F32, tag="po")
                        if not fast:
                            nc.vector.memset(po[:, :], 0.0)
                        nks = len(ks)

                        def emit_av(k, idx, runs, pt):
                            for h in range(2):
                                for ja, jb in runs:
                                    nc.tensor.matmul(
                                        po[0:HD + 1,
                                           h * QCW + ja * ST:
                                           h * QCW + (jb + 1) * ST],
                                        vb[:, k, 2 * g + h, :],
                                        pt[:, h * QCW + ja * ST:
                                           h * QCW + (jb + 1) * ST],
                                        start=(fast and idx == 0),
                                        stop=(fast and idx == nks - 1),
                                        skip_group_check=True)

                        pending = None
                        for idx, k in enumerate(ks):
                            runs = _runs(vjm[k])
                            psc = pscores.tile([ST, 2 * QCW], F32, tag="psc")
                            for ja, jb in runs:
                                for h in range(2):
                                    nc.tensor.matmul(
                                        psc[:, h * QCW + ja * ST:
                                            h * QCW + (jb + 1) * ST],
                                        kt[g][h * HD:(h + 1) * HD,
                                              k * ST:(k + 1) * ST],
                                        qt[g][h * HD:(h + 1) * HD,
                                              qc * QCW + ja * ST:
                                              qc * QCW + (jb + 1) * ST],
                                        start=True, stop=True)
                            if pending is not None:
                                emit_av(*pending)
                            pt = ppt.tile([ST, 2 * QCW], BF16, tag="pt")
                            nc.scalar.activation(
                                pt[:, :], psc[:, :],
                                mybir.ActivationFunctionType.Exp,
                                scale=1.0 / float(np.sqrt(HD)))
                            for j in vjm[k]:
                                if cls[qc * QCT + j, k] == 2:
                                    m = mixed_idx[(qc * QCT + j, k)]
                                    for h in range(2):
                                        nc.vector.tensor_mul(
                                            pt[:, h * QCW + j * ST:
                                               h * QCW + (j + 1) * ST],
                                            pt[:, h * QCW + j * ST:
                                               h * QCW + (j + 1) * ST],
                                            mtb[:, m, :])
                            pending = (k, idx, runs, pt)
                            next(em, None)
                            next(em, None)
                        emit_av(*pending)
                        # finalize (g, qc): free po quickly via a ScalarE
                        # copy, then normalize out of SBUF.
                        sfin = pfin.tile([HD + 1, 2 * QCW], F32, tag="sfin")
                        nc.scalar.copy(sfin[:, :], po[:, :])
                        ltmp = pfin.tile([1, 2 * QCW], F32, tag="ltmp")
                        nc.vector.tensor_copy(ltmp[:, :], sfin[HD:HD + 1, :])
                        rec = pfin.tile([1, 2 * QCW], F32, tag="rec")
                        nc.vector.reciprocal_approx_fast(rec[:, :], ltmp[:, :])
                        rbs = pfin.tile([HD, 2 * QCW], F32, tag="rbs")
                        nc.gpsimd.partition_broadcast(
                            rbs[:, :], rec[0:1, :], channels=HD)
                        for h in range(2):
                            nc.vector.tensor_mul(
                                cat[g][h * HD:(h + 1) * HD,
                                       qc * QCW:(qc + 1) * QCW],
                                sfin[0:HD, h * QCW:(h + 1) * QCW],
                                rbs[:, h * QCW:(h + 1) * QCW])
                    for _ in em:
                        pass

            # ---- epoch C: partial fc_out --------------------------------
            with (
                tc.tile_pool(name="pfc", bufs=2, space="PSUM") as pfc,
                tc.tile_pool(name="pfcs", bufs=3) as pfcs,
            ):
                for jt in range(NQT):
                    py = pfc.tile([ST, D], F32, tag="py")
                    for g in range(NG):
                        for n in range(2):
                            nc.tensor.matmul(
                                py[:, n * 512:(n + 1) * 512],
                                cat[g][:, jt * ST:(jt + 1) * ST],
                                wob[:, g, n * 512:(n + 1) * 512],
                                start=(g == 0), stop=(g == NG - 1))
                    ysb = pfcs.tile([ST, D], BF16, tag="ysb")
                    nc.vector.tensor_add(ysb[:, :], py[:, :], bob[:, :])
                    nc.sync.dma_start(
                        out_d.ap()[jt * ST:(jt + 1) * ST, :], ysb[:, :])

    nc.compile()
    return nc


_CACHE = {}
LAST_RESULT = None


def _get_program(mask):
    key = mask.tobytes()
    if key not in _CACHE:
        cls, mixed = _classify(mask)
        _CACHE[key] = (_build(cls, mixed, max(len(mixed), 1)), cls, mixed)
    return _CACHE[key]


def kernel(x, mask, Wq, bq, Wk, bk, Wv, bv, Wo, bo):
    x = np.asarray(x, dtype=np.float32)
    mask = np.asarray(mask)
    Wq = np.asarray(Wq, dtype=np.float32)
    Wk = np.asarray(Wk, dtype=np.float32)
    Wv = np.asarray(Wv, dtype=np.float32)
    Wo = np.asarray(Wo, dtype=np.float32)
    nc, cls, mixed = _get_program(mask)

    n_maskt = max(len(mixed), 1)
    mt = np.zeros((n_maskt, ST, ST), dtype=BF)
    for i, (j, k) in enumerate(mixed):
        blk = mask[j * ST:(j + 1) * ST, k * ST:(k + 1) * ST]
        mt[i] = (blk != 0).T.astype(BF)

    def pack_pair(W, hh):
        # [128, NDC, NG, 128]: [p, c, g, m*64+e] = W[8hh + 2g+m, 128c+p, e]
        Wl = W[hh * HL:(hh + 1) * HL].reshape(NG, 2, NDC, ST, HD)
        return np.ascontiguousarray(
            Wl.transpose(3, 2, 0, 1, 4).reshape(ST, NDC, NG, ST).astype(BF))

    in_maps = []
    for c in range(N_CORES):
        b, hh = c // 2, c % 2
        Wvl = Wv[hh * HL:(hh + 1) * HL].reshape(HL, NDC, ST, HD)
        wvb = np.ascontiguousarray(
            Wvl.transpose(2, 1, 0, 3).reshape(ST, NDC, HL * HD).astype(BF))
        Wol = Wo[hh * HL * HD:(hh + 1) * HL * HD].reshape(NG, 2, HD, D)
        wob = np.ascontiguousarray(
            Wol.transpose(1, 2, 0, 3).reshape(ST, NG, D).astype(BF))
        bql = np.asarray(bq, dtype=np.float32)[hh * HL:(hh + 1) * HL]
        bkl = np.asarray(bk, dtype=np.float32)[hh * HL:(hh + 1) * HL]
        bqp = np.ascontiguousarray(
            bql.reshape(NG, 2, HD).transpose(1, 2, 0).reshape(ST, NG))
        bkp = np.ascontiguousarray(
            bkl.reshape(NG, 2, HD).transpose(1, 2, 0).reshape(ST, NG))
        m = {
            "x": np.ascontiguousarray(x[b].astype(BF)),
            "wqp": pack_pair(Wq, hh),
            "wkp": pack_pair(Wk, hh),
            "wvb": wvb,
            "wob": wob,
            "bqp": bqp,
            "bkp": bkp,
            "bvf": np.ascontiguousarray(
                np.asarray(bv, dtype=np.float32)[hh * HL:(hh + 1) * HL]),
            "bob": (np.asarray(bo, dtype=np.float32) if hh == 0
                    else np.zeros(D, dtype=np.float32)),
            "maskt": mt,
        }
        in_maps.append(m)

    res = run_bass_kernel_spmd(
        nc, in_maps, core_ids=list(range(N_CORES)),
        trace=os.environ.get("BASS_KERNEL_TRACE", "0") == "1")
    global LAST_RESULT
    LAST_RESULT = res

    out = np.empty((B, S, D), dtype=np.float32)
    for b in range(B):
        out[b] = (res.results[2 * b]["out"].astype(np.float32)
                  + res.results[2 * b + 1]["out"].astype(np.float32))
    return out


# revision 12
# speedup vs baseline: 1.6923x; 1.0663x over previous
"""Trainium2 Bass kernel: causal multi-head attention (B=4,S=2048,D=1024,H=16).

Sharding (8 cores, host-side pair reduction): core c -> batch b=c//2,
head-half hh=c%2 (local heads hh*8..hh*8+7, i.e. 4 head pairs).  Each core
computes Q/K/V for its 8 heads over ALL 2048 rows, full causal attention,
and a PARTIAL fc_out against the row-shard Wo[hh*512:(hh+1)*512].  The host
sums the two partials per batch (the "all-reduce" of the row-sharded Wo).

Device pipeline per core (all matmuls bf16, f32 accumulation):
  - x^T arrives directly via 8 DMA-xbar transposes from DRAM (no PE
    transposes, no row-major staging).
  - Attention (the ScalarE exp stream is the pacer): per pair g, per
    q-chunk of 512 cols, per k-tile: scores^T pair = two row-tiled
    concurrent matmuls (heads at array rows 0-63 / 64-127) -> one
    1024-wide exp on ScalarE (scale folded, PSUM->SBUF bf16), 0/1 mask
    multiply on diag blocks, ones-augmented AV accumulation one k-step
    behind (row 64 = softmax denominator).
  - All other PE work (V projections, K^T/Q^T of later pairs, fc_out
    tiles) is emitted as "filler" chunks pulled into the exp-wait gaps,
    gated by markers so the in-order PE queue can never deadlock.
  - Finalize per (g, q-chunk): free po via a DVE copy, reciprocal of the
    denominators, GpSimd partition-broadcast, normalize into cat (bf16).
  - fc_out tiles run as filler during the last pair; bf16 output.

Weights are pre-packed on the host into the exact stationary layouts
(bf16).  The program is specialized at build time to the mask's 128x128
block structure (computed from the actual mask input, so it stays correct
for any mask).
"""

import os
import numpy as np
import ml_dtypes

import concourse.bass as bass
import concourse.mybir as mybir
import concourse.tile as tile
from concourse import bacc
from concourse.bass_utils import run_bass_kernel_spmd

B, S, D, H, HD = 4, 2048, 1024, 16, 64
N_CORES = 8
ST = 128                 # tile edge
NKT = S // ST            # 16 k tiles
NQT = S // ST            # 16 q tiles
NDC = D // ST            # 8 contraction chunks
HL = H // 2              # 8 local heads per core
NG = HL // 2             # 4 local head pairs
NQC = 4                  # q chunks per core
QCW = S // NQC           # 512 cols per q chunk (4 q tiles)
QCT = QCW // ST          # 4 q tiles per chunk

F32 = mybir.dt.float32
BF16 = mybir.dt.bfloat16
BF = ml_dtypes.bfloat16


def _classify(mask: np.ndarray):
    """128x128 block structure of the mask: 0 skip, 1 full, 2 mixed."""
    cls = np.zeros((NQT, NKT), dtype=int)
    for j in range(NQT):
        for k in range(NKT):
            blk = mask[j * ST:(j + 1) * ST, k * ST:(k + 1) * ST]
            if (blk != 0).all():
                cls[j, k] = 1
            elif (blk == 0).all():
                cls[j, k] = 0
            else:
                cls[j, k] = 2
    mixed = [(j, k) for j in range(NQT) for k in range(NKT) if cls[j, k] == 2]
    return cls, mixed


def _runs(valid):
    """Contiguous runs [(ja, jb)] of a sorted list of chunk-local j."""
    runs = []
    for j in valid:
        if runs and j == runs[-1][1] + 1:
            runs[-1][1] = j
        else:
            runs.append([j, j])
    return [(a, b) for a, b in runs]


class Filler:
    """Ordered queue of PE-work chunks with tags (drain points) and gates."""

    def __init__(self):
        self.q = []           # (tag, gate, fn)
        self.open = set()
        self.emitted = set()

    def add(self, fn, tag=None, gate=None):
        self.q.append((tag, gate, fn))

    def open_gate(self, gate):
        self.open.add(gate)

    def _emit_front(self):
        tag, gate, fn = self.q.pop(0)
        fn()
        if tag:
            self.emitted.add(tag)
        return tag

    def pull(self, n=1):
        for _ in range(n):
            if not self.q:
                return
            tag, gate, fn = self.q[0]
            if gate is not None and gate not in self.open:
                return
            self._emit_front()

    def drain(self, tag):
        if tag in self.emitted:
            return
        while self.q:
            g = self.q[0][1]
            assert g is None or g in self.open, f"drain past closed gate {g}"
            if self._emit_front() == tag:
                return
        raise KeyError(tag)

    def drain_all(self):
        while self.q:
            self._emit_front()


def _build(cls, mixed, n_maskt):
    nc = bacc.Bacc("TRN2", target_bir_lowering=False, debug=False,
                   num_devices=N_CORES)

    x_d = nc.dram_tensor("x", [S, D], BF16, kind="ExternalInput")
    wqp_d = nc.dram_tensor("wqp", [ST, NDC, NG, ST], BF16, kind="ExternalInput")
    wkp_d = nc.dram_tensor("wkp", [ST, NDC, NG, ST], BF16, kind="ExternalInput")
    wvb_d = nc.dram_tensor("wvb", [ST, NDC, HL * HD], BF16, kind="ExternalInput")
    wob_d = nc.dram_tensor("wob", [ST, NG, D], BF16, kind="ExternalInput")
    bqp_d = nc.dram_tensor("bqp", [ST, NG], F32, kind="ExternalInput")
    bkp_d = nc.dram_tensor("bkp", [ST, NG], F32, kind="ExternalInput")
    bvf_d = nc.dram_tensor("bvf", [HL, HD], F32, kind="ExternalInput")
    bob_d = nc.dram_tensor("bob", [D], F32, kind="ExternalInput")
    mt_d = nc.dram_tensor("maskt", [n_maskt, ST, ST], BF16, kind="ExternalInput")
    out_d = nc.dram_tensor("out", [S, D], BF16, kind="ExternalOutput")

    mixed_idx = {jk: i for i, jk in enumerate(mixed)}

    chunk_ks, chunk_vj = [], []
    for qc in range(NQC):
        vj = {}
        for k in range(NKT):
            v = [j for j in range(QCT) if cls[qc * QCT + j, k]]
            if v:
                vj[k] = v
        chunk_ks.append(sorted(vj))
        chunk_vj.append(vj)

    with tile.TileContext(nc) as tc:
        with tc.tile_pool(name="pp", bufs=1) as pp:
            # ---- persistent SBUF ----------------------------------------
            kt = [pp.tile([ST, S], BF16, name=f"kt{g}", tag=f"kt{g}")
                  for g in range(NG)]
            qt = [pp.tile([ST, S], BF16, name=f"qt{g}", tag=f"qt{g}")
                  for g in range(NG)]
            cat = [pp.tile([ST, S], BF16, name=f"cat{g}", tag=f"cat{g}")
                   for g in range(NG)]
            xt = [pp.tile([ST, S], BF16, name=f"xt{c}", tag=f"xt{c}")
                  for c in range(NDC)]
            vb = pp.tile([ST, NKT, HL, HD + 1], BF16, name="vb", tag="vb")
            wqp = pp.tile([ST, NDC, NG, ST], BF16, name="wqp", tag="wqp")
            wkp = pp.tile([ST, NDC, NG, ST], BF16, name="wkp", tag="wkp")
            wvb = pp.tile([ST, NDC, HL * HD], BF16, name="wvb", tag="wvb")
            wob = pp.tile([ST, NG, D], BF16, name="wob", tag="wob")
            bqp = pp.tile([ST, NG], F32, name="bqp", tag="bqp")
            bkp = pp.tile([ST, NG], F32, name="bkp", tag="bkp")
            bvf = pp.tile([ST, HL, HD], F32, name="bvf", tag="bvf")
            bob = pp.tile([ST, D], F32, name="bob", tag="bob")
            mtb = pp.tile([ST, max(n_maskt, 1), ST], BF16, name="mtb", tag="mtb")

            # x^T via DMA-xbar transposes (single queue: the xbar engine
            # appears to be a shared resource, so keep them serialized).
            for c in range(NDC):
                nc.sync.dma_start_transpose(xt[c][:, :],
                                            x_d.ap()[:, c * ST:(c + 1) * ST])
            # weights on the gpsimd (SWDGE) queue, most-urgent first
            nc.gpsimd.dma_start(wkp[:, :, :, :], wkp_d.ap())
            nc.gpsimd.dma_start(wvb[:, :, :], wvb_d.ap())
            nc.gpsimd.dma_start(wqp[:, :, :, :], wqp_d.ap())
            nc.gpsimd.dma_start(mtb[:, :, :],
                                mt_d.ap().rearrange("m p f -> p m f"))
            nc.gpsimd.dma_start(wob[:, :, :], wob_d.ap())
            # small tensors after the x chunks on the HWDGE queues
            nc.scalar.dma_start(bqp[:, :], bqp_d.ap())
            nc.scalar.dma_start(bkp[:, :], bkp_d.ap())
            src = bvf_d.ap()
            nc.scalar.dma_start(
                bvf[:, :, :],
                bass.AP(tensor=src.tensor, offset=src.offset,
                        ap=[[0, ST]] + list(src.ap)))
            src = bob_d.ap()
            nc.scalar.dma_start(
                bob[:, :],
                bass.AP(tensor=src.tensor, offset=src.offset,
                        ap=[[0, ST]] + list(src.ap)))

            nc.vector.memset(vb[:, :, :, HD:HD + 1], 1.0)

            with (
                tc.tile_pool(name="ppsc", bufs=2, space="PSUM") as ppsc,
                tc.tile_pool(name="ppo", bufs=1, space="PSUM") as ppo,
                tc.tile_pool(name="ppv", bufs=2, space="PSUM") as ppv,
                tc.tile_pool(name="ppt", bufs=3) as ppt,
                tc.tile_pool(name="pfin", bufs=2) as pfin,
                tc.tile_pool(name="pfcs", bufs=3) as pfcs,
            ):
                def emit_v(st):
                    psv = ppv.tile([ST, HL * HD], F32, tag="pv")
                    for c in range(NDC):
                        nc.tensor.matmul(
                            psv[:, :], xt[c][:, st * ST:(st + 1) * ST],
                            wvb[:, c, :], start=(c == 0), stop=(c == NDC - 1),
                            skip_group_check=True)
                    nc.vector.tensor_add(
                        vb[:, st, :, 0:HD],
                        psv[:, :].rearrange("p (h e) -> p h e", h=HL),
                        bvf[:, :, :])

                def emit_kq(g, sg, which):
                    w_t, bias_t, dst = ((wkp, bkp, kt[g]) if which == 0
                                        else (wqp, bqp, qt[g]))
                    ps = ppv.tile([ST, 512], F32, tag="pv")
                    for c in range(NDC):
                        nc.tensor.matmul(
                            ps[:, :], w_t[:, c, g, :],
                            xt[c][:, sg * 512:(sg + 1) * 512],
                            start=(c == 0), stop=(c == NDC - 1),
                            skip_group_check=True)
                    nc.vector.tensor_scalar(
                        dst[:, sg * 512:(sg + 1) * 512], ps[:, :],
                        bias_t[:, g:g + 1], None, mybir.AluOpType.add)

                def emit_fc(jt):
                    py = [ppv.tile([ST, 512], F32, tag="pv", name=f"py{n}")
                          for n in range(2)]
                    for g in range(NG):
                        for n in range(2):
                            nc.tensor.matmul(
                                py[n][:, :],
                                cat[g][:, jt * ST:(jt + 1) * ST],
                                wob[:, g, n * 512:(n + 1) * 512],
                                start=(g == 0), stop=(g == NG - 1),
                                skip_group_check=True)
                    ysb = pfcs.tile([ST, D], BF16, tag="ysb")
                    for n in range(2):
                        nc.vector.tensor_add(ysb[:, n * 512:(n + 1) * 512],
                                             py[n][:, :],
                                             bob[:, n * 512:(n + 1) * 512])
                    eng = nc.sync if jt % 2 == 0 else nc.scalar
                    eng.dma_start(out_d.ap()[jt * ST:(jt + 1) * ST, :],
                                  ysb[:, :])

                # ---- preamble: critical path to the first exp ----------
                emit_kq(0, 0, 0)
                emit_kq(0, 0, 1)
                for st in range(4):
                    emit_v(st)

                # ---- filler queue --------------------------------------
                fil = Filler()
                for blk in range(1, 4):
                    for st in range(4 * blk, 4 * blk + 4):
                        fil.add(lambda st=st: emit_v(st), tag=f"v{st}")
                    fil.add(lambda blk=blk: emit_kq(0, blk, 0))
                    fil.add(lambda blk=blk: emit_kq(0, blk, 1),
                            tag=f"kq0s{blk}")
                for g in range(1, NG):
                    for sg in range(4):
                        fil.add(lambda g=g, sg=sg: emit_kq(g, sg, 0))
                        fil.add(lambda g=g, sg=sg: emit_kq(g, sg, 1))
                    fil.add(lambda: None, tag=f"pair{g}")
                for jt in range(NQT):
                    fil.add(lambda jt=jt: emit_fc(jt), tag=f"fc{jt}",
                            gate=f"cat_qc{jt // QCT}")

                # ---- attention (exp-paced), filler in the gaps ---------
                for g in range(NG):
                    if g > 0:
                        fil.drain(f"pair{g}")
                    for qc in range(NQC):
                        if g == 0 and qc > 0:
                            fil.drain(f"kq0s{qc}")
                        ks = chunk_ks[qc]
                        vjm = chunk_vj[qc]
                        if not ks:
                            continue
                        union = sorted({j for v in vjm.values() for j in v})
                        fast = vjm[ks[0]] == union
                        po = ppo.tile([HD + 1, 2 * QCW], F32, tag="po")
                        if not fast:
                            nc.vector.memset(po[:, :], 0.0)
                        nks = len(ks)

                        def emit_av(k, idx, runs, pt):
                            for h in range(2):
                                for ja, jb in runs:
                                    nc.tensor.matmul(
                                        po[0:HD + 1,
                                           h * QCW + ja * ST:
                                           h * QCW + (jb + 1) * ST],
                                        vb[:, k, 2 * g + h, :],
                                        pt[:, h * QCW + ja * ST:
                                           h * QCW + (jb + 1) * ST],
                                        start=(fast and idx == 0),
                                        stop=(fast and idx == nks - 1),
                                        skip_group_check=True)

                        pending = None
                        for idx, k in enumerate(ks):
                            runs = _runs(vjm[k])
                            psc = ppsc.tile([ST, 2 * QCW], F32, tag="psc")
                            for ja, jb in runs:
                                for h in range(2):
                                    nc.tensor.matmul(
                                        psc[:, h * QCW + ja * ST:
                                            h * QCW + (jb + 1) * ST],
                                        kt[g][h * HD:(h + 1) * HD,
                                              k * ST:(k + 1) * ST],
                                        qt[g][h * HD:(h + 1) * HD,
                                              qc * QCW + ja * ST:
                                              qc * QCW + (jb + 1) * ST],
                                        start=True, stop=True)
                            if pending is not None:
                                emit_av(*pending)
                            pt = ppt.tile([ST, 2 * QCW], BF16, tag="pt")
                            nc.scalar.activation(
                                pt[:, :], psc[:, :],
                                mybir.ActivationFunctionType.Exp,
                                scale=1.0 / float(np.sqrt(HD)))
                            for j in vjm[k]:
                                if cls[qc * QCT + j, k] == 2:
                                    m = mixed_idx[(qc * QCT + j, k)]
                                    for h in range(2):
                                        nc.vector.tensor_mul(
                                            pt[:, h * QCW + j * ST:
                                               h * QCW + (j + 1) * ST],
                                            pt[:, h * QCW + j * ST:
                                               h * QCW + (j + 1) * ST],
                                            mtb[:, m, :])
                            pending = (k, idx, runs, pt)
                            fil.pull(1)
                        emit_av(*pending)
                        # finalize (g, qc): free po via a DVE copy, then
                        # normalize out of SBUF.
                        sfin = pfin.tile([HD + 1, 2 * QCW], F32, tag="sfin")
                        nc.vector.tensor_copy(sfin[:, :], po[:, :])
                        ltmp = pfin.tile([1, 2 * QCW], F32, tag="ltmp")
                        nc.vector.tensor_copy(ltmp[:, :], sfin[HD:HD + 1, :])
                        rec = pfin.tile([1, 2 * QCW], F32, tag="rec")
                        nc.vector.reciprocal_approx_fast(rec[:, :], ltmp[:, :])
                        rbs = pfin.tile([HD, 2 * QCW], F32, tag="rbs")
                        nc.gpsimd.partition_broadcast(
                            rbs[:, :], rec[0:1, :], channels=HD)
                        for h in range(2):
                            nc.vector.tensor_mul(
                                cat[g][h * HD:(h + 1) * HD,
                                       qc * QCW:(qc + 1) * QCW],
                                sfin[0:HD, h * QCW:(h + 1) * QCW],
                                rbs[:, h * QCW:(h + 1) * QCW])
                        if g == NG - 1:
                            fil.open_gate(f"cat_qc{qc}")
                fil.drain_all()

    nc.compile()
    return nc


_CACHE = {}
LAST_RESULT = None


def _get_program(mask):
    key = mask.tobytes()
    if key not in _CACHE:
        cls, mixed = _classify(mask)
        _CACHE[key] = (_build(cls, mixed, max(len(mixed), 1)), cls, mixed)
    return _CACHE[key]


def kernel(x, mask, Wq, bq, Wk, bk, Wv, bv, Wo, bo):
    x = np.asarray(x, dtype=np.float32)
    mask = np.asarray(mask)
    Wq = np.asarray(Wq, dtype=np.float32)
    Wk = np.asarray(Wk, dtype=np.float32)
    Wv = np.asarray(Wv, dtype=np.float32)
    Wo = np.asarray(Wo, dtype=np.float32)
    nc, cls, mixed = _get_program(mask)

    n_maskt = max(len(mixed), 1)
    mt = np.zeros((n_maskt, ST, ST), dtype=BF)
    for i, (j, k) in enumerate(mixed):
        blk = mask[j * ST:(j + 1) * ST, k * ST:(k + 1) * ST]
        mt[i] = (blk != 0).T.astype(BF)

    def pack_pair(W, hh):
        # [128, NDC, NG, 128]: [p, c, g, m*64+e] = W[8hh + 2g+m, 128c+p, e]
        Wl = W[hh * HL:(hh + 1) * HL].reshape(NG, 2, NDC, ST, HD)
        return np.ascontiguousarray(
            Wl.transpose(3, 2, 0, 1, 4).reshape(ST, NDC, NG, ST).astype(BF))

    in_maps = []
    for c in range(N_CORES):
        b, hh = c // 2, c % 2
        Wvl = Wv[hh * HL:(hh + 1) * HL].reshape(HL, NDC, ST, HD)
        wvb = np.ascontiguousarray(
            Wvl.transpose(2, 1, 0, 3).reshape(ST, NDC, HL * HD).astype(BF))
        Wol = Wo[hh * HL * HD:(hh + 1) * HL * HD].reshape(NG, 2, HD, D)
        wob = np.ascontiguousarray(
            Wol.transpose(1, 2, 0, 3).reshape(ST, NG, D).astype(BF))
        bql = np.asarray(bq, dtype=np.float32)[hh * HL:(hh + 1) * HL]
        bkl = np.asarray(bk, dtype=np.float32)[hh * HL:(hh + 1) * HL]
        bqp = np.ascontiguousarray(
            bql.reshape(NG, 2, HD).transpose(1, 2, 0).reshape(ST, NG))
        bkp = np.ascontiguousarray(
            bkl.reshape(NG, 2, HD).transpose(1, 2, 0).reshape(ST, NG))
        m = {
            "x": np.ascontiguousarray(x[b].astype(BF)),
            "wqp": pack_pair(Wq, hh),
            "wkp": pack_pair(Wk, hh),
            "wvb": wvb,
            "wob": wob,
            "bqp": bqp,
            "bkp": bkp,
            "bvf": np.ascontiguousarray(
                np.asarray(bv, dtype=np.float32)[hh * HL:(hh + 1) * HL]),
            "bob": (np.asarray(bo, dtype=np.float32) if hh == 0
                    else np.zeros(D, dtype=np.float32)),
            "maskt": mt,
        }
        in_maps.append(m)

    res = run_bass_kernel_spmd(
        nc, in_maps, core_ids=list(range(N_CORES)),
        trace=os.environ.get("BASS_KERNEL_TRACE", "0") == "1")
    global LAST_RESULT
    LAST_RESULT = res

    out = np.empty((B, S, D), dtype=np.float32)
    for b in range(B):
        out[b] = (res.results[2 * b]["out"].astype(np.float32)
                  + res.results[2 * b + 1]["out"].astype(np.float32))
    return out


# revision 13
# speedup vs baseline: 1.7472x; 1.0325x over previous
"""Trainium2 Bass kernel: causal multi-head attention (B=4,S=2048,D=1024,H=16).

Sharding (8 cores, host-side pair reduction): core c -> batch b=c//2,
head-half hh=c%2 (local heads hh*8..hh*8+7, i.e. 4 head pairs).  Each core
computes Q/K/V for its 8 heads over ALL 2048 rows, full causal attention,
and a PARTIAL fc_out against the row-shard Wo[hh*512:(hh+1)*512].  The host
sums the two partials per batch (the "all-reduce" of the row-sharded Wo).

Device pipeline per core (all matmuls bf16, f32 accumulation):
  - x^T arrives directly via 8 DMA-xbar transposes from DRAM (no PE
    transposes, no row-major staging).
  - Attention (the ScalarE exp stream is the pacer): per pair g, per
    q-chunk of 512 cols, per k-tile: scores^T pair = two row-tiled
    concurrent matmuls (heads at array rows 0-63 / 64-127) -> one
    1024-wide exp on ScalarE (scale folded, PSUM->SBUF bf16), 0/1 mask
    multiply on diag blocks, ones-augmented AV accumulation one k-step
    behind (row 64 = softmax denominator).
  - All other PE work (V projections, K^T/Q^T of later pairs, fc_out
    tiles) is emitted as "filler" chunks pulled into the exp-wait gaps,
    gated by markers so the in-order PE queue can never deadlock.
  - Finalize per (g, q-chunk): free po via a DVE copy, reciprocal of the
    denominators, GpSimd partition-broadcast, normalize into cat (bf16).
  - fc_out tiles run as filler during the last pair; bf16 output.

Weights are pre-packed on the host into the exact stationary layouts
(bf16).  The program is specialized at build time to the mask's 128x128
block structure (computed from the actual mask input, so it stays correct
for any mask).
"""

import os
import numpy as np
import ml_dtypes

import concourse.bass as bass
import concourse.mybir as mybir
import concourse.tile as tile
from concourse import bacc
from concourse.bass_utils import run_bass_kernel_spmd
from concourse.masks import make_identity

B, S, D, H, HD = 4, 2048, 1024, 16, 64
N_CORES = 8
ST = 128                 # tile edge
NKT = S // ST            # 16 k tiles
NQT = S // ST            # 16 q tiles
NDC = D // ST            # 8 contraction chunks
HL = H // 2              # 8 local heads per core
NG = HL // 2             # 4 local head pairs
NQC = 4                  # q chunks per core
QCW = S // NQC           # 512 cols per q chunk (4 q tiles)
QCT = QCW // ST          # 4 q tiles per chunk

F32 = mybir.dt.float32
BF16 = mybir.dt.bfloat16
BF = ml_dtypes.bfloat16


def _classify(mask: np.ndarray):
    """128x128 block structure of the mask: 0 skip, 1 full, 2 mixed."""
    cls = np.zeros((NQT, NKT), dtype=int)
    for j in range(NQT):
        for k in range(NKT):
            blk = mask[j * ST:(j + 1) * ST, k * ST:(k + 1) * ST]
            if (blk != 0).all():
                cls[j, k] = 1
            elif (blk == 0).all():
                cls[j, k] = 0
            else:
                cls[j, k] = 2
    mixed = [(j, k) for j in range(NQT) for k in range(NKT) if cls[j, k] == 2]
    return cls, mixed


def _runs(valid):
    """Contiguous runs [(ja, jb)] of a sorted list of chunk-local j."""
    runs = []
    for j in valid:
        if runs and j == runs[-1][1] + 1:
            runs[-1][1] = j
        else:
            runs.append([j, j])
    return [(a, b) for a, b in runs]


class Filler:
    """Ordered queue of PE-work chunks with tags (drain points) and gates."""

    def __init__(self):
        self.q = []           # (tag, gate, fn)
        self.open = set()
        self.emitted = set()

    def add(self, fn, tag=None, gate=None):
        self.q.append((tag, gate, fn))

    def open_gate(self, gate):
        self.open.add(gate)

    def _emit_front(self):
        tag, gate, fn = self.q.pop(0)
        fn()
        if tag:
            self.emitted.add(tag)
        return tag

    def pull(self, n=1):
        for _ in range(n):
            if not self.q:
                return
            tag, gate, fn = self.q[0]
            if gate is not None and gate not in self.open:
                return
            self._emit_front()

    def drain(self, tag):
        if tag in self.emitted:
            return
        while self.q:
            g = self.q[0][1]
            assert g is None or g in self.open, f"drain past closed gate {g}"
            if self._emit_front() == tag:
                return
        raise KeyError(tag)

    def drain_all(self):
        while self.q:
            self._emit_front()


def _build(cls, mixed, n_maskt):
    nc = bacc.Bacc("TRN2", target_bir_lowering=False, debug=False,
                   num_devices=N_CORES)

    x_d = nc.dram_tensor("x", [S, D], BF16, kind="ExternalInput")
    wqp_d = nc.dram_tensor("wqp", [ST, NDC, NG, ST], BF16, kind="ExternalInput")
    wkp_d = nc.dram_tensor("wkp", [ST, NDC, NG, ST], BF16, kind="ExternalInput")
    wvb_d = nc.dram_tensor("wvb", [ST, NDC, HL * HD], BF16, kind="ExternalInput")
    wob_d = nc.dram_tensor("wob", [ST, NG, D], BF16, kind="ExternalInput")
    bqp_d = nc.dram_tensor("bqp", [ST, NG], F32, kind="ExternalInput")
    bkp_d = nc.dram_tensor("bkp", [ST, NG], F32, kind="ExternalInput")
    bvf_d = nc.dram_tensor("bvf", [HL, HD], F32, kind="ExternalInput")
    bob_d = nc.dram_tensor("bob", [D], F32, kind="ExternalInput")
    mt_d = nc.dram_tensor("maskt", [n_maskt, ST, ST], BF16, kind="ExternalInput")
    out_d = nc.dram_tensor("out", [S, D], BF16, kind="ExternalOutput")

    mixed_idx = {jk: i for i, jk in enumerate(mixed)}

    chunk_ks, chunk_vj = [], []
    for qc in range(NQC):
        vj = {}
        for k in range(NKT):
            v = [j for j in range(QCT) if cls[qc * QCT + j, k]]
            if v:
                vj[k] = v
        chunk_ks.append(sorted(vj))
        chunk_vj.append(vj)

    with tile.TileContext(nc) as tc:
        with tc.tile_pool(name="pp", bufs=1) as pp:
            # ---- persistent SBUF ----------------------------------------
            kt = [pp.tile([ST, S], BF16, name=f"kt{g}", tag=f"kt{g}")
                  for g in range(NG)]
            qt = [pp.tile([ST, S], BF16, name=f"qt{g}", tag=f"qt{g}")
                  for g in range(NG)]
            cat = [pp.tile([ST, S], BF16, name=f"cat{g}", tag=f"cat{g}")
                   for g in range(NG)]
            xt = [pp.tile([ST, S], BF16, name=f"xt{c}", tag=f"xt{c}")
                  for c in range(NDC)]
            vb = pp.tile([ST, NKT, HL, HD + 1], BF16, name="vb", tag="vb")
            wqp = pp.tile([ST, NDC, NG, ST], BF16, name="wqp", tag="wqp")
            wkp = pp.tile([ST, NDC, NG, ST], BF16, name="wkp", tag="wkp")
            wvb = pp.tile([ST, NDC, HL * HD], BF16, name="wvb", tag="wvb")
            wob = pp.tile([ST, NG, D], BF16, name="wob", tag="wob")
            bqp = pp.tile([ST, NG], F32, name="bqp", tag="bqp")
            bkp = pp.tile([ST, NG], F32, name="bkp", tag="bkp")
            bvf = pp.tile([ST, HL, HD], F32, name="bvf", tag="bvf")
            bob = pp.tile([ST, D], F32, name="bob", tag="bob")
            mtb = pp.tile([ST, max(n_maskt, 1), ST], BF16, name="mtb", tag="mtb")
            ident = pp.tile([ST, ST], BF16, name="ident", tag="ident")

            # weights on the gpsimd (SWDGE) queue, most-urgent first
            nc.gpsimd.dma_start(wkp[:, :, :, :], wkp_d.ap())
            nc.gpsimd.dma_start(wvb[:, :, :], wvb_d.ap())
            nc.gpsimd.dma_start(wqp[:, :, :, :], wqp_d.ap())
            nc.gpsimd.dma_start(mtb[:, :, :],
                                mt_d.ap().rearrange("m p f -> p m f"))
            nc.gpsimd.dma_start(wob[:, :, :], wob_d.ap())
            # small tensors after the x chunks on the HWDGE queues
            nc.scalar.dma_start(bqp[:, :], bqp_d.ap())
            nc.scalar.dma_start(bkp[:, :], bkp_d.ap())
            src = bvf_d.ap()
            nc.scalar.dma_start(
                bvf[:, :, :],
                bass.AP(tensor=src.tensor, offset=src.offset,
                        ap=[[0, ST]] + list(src.ap)))
            src = bob_d.ap()
            nc.scalar.dma_start(
                bob[:, :],
                bass.AP(tensor=src.tensor, offset=src.offset,
                        ap=[[0, ST]] + list(src.ap)))

            nc.vector.memset(vb[:, :, :, HD:HD + 1], 1.0)
            make_identity(nc, ident[:, :])

            # ---- ramp: x^T via PE transposes, K0/Q0 sg0, V st0-3 ------
            with (
                tc.tile_pool(name="pxb", bufs=4) as pxb,
                tc.tile_pool(name="ppst", bufs=3, space="PSUM") as ppst,
                tc.tile_pool(name="ppvr", bufs=2, space="PSUM") as ppvr,
            ):
                def emit_v_ramp(st):
                    psv = ppvr.tile([ST, HL * HD], F32, tag="pvr")
                    for c in range(NDC):
                        nc.tensor.matmul(
                            psv[:, :], xt[c][:, st * ST:(st + 1) * ST],
                            wvb[:, c, :], start=(c == 0), stop=(c == NDC - 1),
                            skip_group_check=True)
                    nc.vector.tensor_add(
                        vb[:, st, :, 0:HD],
                        psv[:, :].rearrange("p (h e) -> p h e", h=HL),
                        bvf[:, :, :])

                def emit_kq_ramp(g, sg, which):
                    w_t, bias_t, dst = ((wkp, bkp, kt[g]) if which == 0
                                        else (wqp, bqp, qt[g]))
                    ps = ppvr.tile([ST, 512], F32, tag="pvr")
                    for c in range(NDC):
                        nc.tensor.matmul(
                            ps[:, :], w_t[:, c, g, :],
                            xt[c][:, sg * 512:(sg + 1) * 512],
                            start=(c == 0), stop=(c == NDC - 1),
                            skip_group_check=True)
                    nc.vector.tensor_scalar(
                        dst[:, sg * 512:(sg + 1) * 512], ps[:, :],
                        bias_t[:, g:g + 1], None, mybir.AluOpType.add)

                for st in range(NKT):
                    xb = pxb.tile([ST, D], BF16, tag="xb")
                    eng = nc.sync if st % 2 == 0 else nc.scalar
                    eng.dma_start(xb[:, :],
                                  x_d.ap()[st * ST:(st + 1) * ST, :])
                    for c in range(NDC):
                        pst = ppst.tile([ST, ST], BF16, tag="pst")
                        nc.tensor.transpose(
                            pst[:, :], xb[:, c * ST:(c + 1) * ST], ident[:, :])
                        nc.scalar.copy(xt[c][:, st * ST:(st + 1) * ST],
                                       pst[:, :])
                    if st == 3:
                        emit_kq_ramp(0, 0, 0)
                        emit_kq_ramp(0, 0, 1)
                        for s0 in range(4):
                            emit_v_ramp(s0)

            with (
                tc.tile_pool(name="ppsc", bufs=2, space="PSUM") as ppsc,
                tc.tile_pool(name="ppo", bufs=1, space="PSUM") as ppo,
                tc.tile_pool(name="ppv", bufs=2, space="PSUM") as ppv,
                tc.tile_pool(name="ppt", bufs=3) as ppt,
                tc.tile_pool(name="pfin", bufs=2) as pfin,
                tc.tile_pool(name="pfcs", bufs=3) as pfcs,
            ):
                def emit_v(st):
                    psv = ppv.tile([ST, HL * HD], F32, tag="pv")
                    for c in range(NDC):
                        nc.tensor.matmul(
                            psv[:, :], xt[c][:, st * ST:(st + 1) * ST],
                            wvb[:, c, :], start=(c == 0), stop=(c == NDC - 1),
                            skip_group_check=True)
                    nc.vector.tensor_add(
                        vb[:, st, :, 0:HD],
                        psv[:, :].rearrange("p (h e) -> p h e", h=HL),
                        bvf[:, :, :])

                def emit_kq(g, sg, which):
                    w_t, bias_t, dst = ((wkp, bkp, kt[g]) if which == 0
                                        else (wqp, bqp, qt[g]))
                    ps = ppv.tile([ST, 512], F32, tag="pv")
                    for c in range(NDC):
                        nc.tensor.matmul(
                            ps[:, :], w_t[:, c, g, :],
                            xt[c][:, sg * 512:(sg + 1) * 512],
                            start=(c == 0), stop=(c == NDC - 1),
                            skip_group_check=True)
                    nc.vector.tensor_scalar(
                        dst[:, sg * 512:(sg + 1) * 512], ps[:, :],
                        bias_t[:, g:g + 1], None, mybir.AluOpType.add)

                def emit_fc(jt):
                    py = [ppv.tile([ST, 512], F32, tag="pv", name=f"py{n}")
                          for n in range(2)]
                    for g in range(NG):
                        for n in range(2):
                            nc.tensor.matmul(
                                py[n][:, :],
                                cat[g][:, jt * ST:(jt + 1) * ST],
                                wob[:, g, n * 512:(n + 1) * 512],
                                start=(g == 0), stop=(g == NG - 1),
                                skip_group_check=True)
                    ysb = pfcs.tile([ST, D], BF16, tag="ysb")
                    for n in range(2):
                        nc.vector.tensor_add(ysb[:, n * 512:(n + 1) * 512],
                                             py[n][:, :],
                                             bob[:, n * 512:(n + 1) * 512])
                    eng = nc.sync if jt % 2 == 0 else nc.scalar
                    eng.dma_start(out_d.ap()[jt * ST:(jt + 1) * ST, :],
                                  ysb[:, :])

                # ---- filler queue --------------------------------------
                fil = Filler()
                for blk in range(1, 4):
                    for st in range(4 * blk, 4 * blk + 4):
                        fil.add(lambda st=st: emit_v(st), tag=f"v{st}")
                    fil.add(lambda blk=blk: emit_kq(0, blk, 0))
                    fil.add(lambda blk=blk: emit_kq(0, blk, 1),
                            tag=f"kq0s{blk}")
                for g in range(1, NG):
                    for sg in range(4):
                        fil.add(lambda g=g, sg=sg: emit_kq(g, sg, 0))
                        fil.add(lambda g=g, sg=sg: emit_kq(g, sg, 1))
                    fil.add(lambda: None, tag=f"pair{g}")
                for jt in range(NQT):
                    fil.add(lambda jt=jt: emit_fc(jt), tag=f"fc{jt}",
                            gate=f"cat_qc{jt // QCT}")

                # ---- attention (exp-paced), filler in the gaps ---------
                for g in range(NG):
                    if g > 0:
                        fil.drain(f"pair{g}")
                    for qc in range(NQC):
                        if g == 0 and qc > 0:
                            fil.drain(f"kq0s{qc}")
                        ks = chunk_ks[qc]
                        vjm = chunk_vj[qc]
                        if not ks:
                            continue
                        union = sorted({j for v in vjm.values() for j in v})
                        fast = vjm[ks[0]] == union
                        po = ppo.tile([HD + 1, 2 * QCW], F32, tag="po")
                        if not fast:
                            nc.vector.memset(po[:, :], 0.0)
                        nks = len(ks)

                        def emit_av(k, idx, runs, pt):
                            for h in range(2):
                                for ja, jb in runs:
                                    nc.tensor.matmul(
                                        po[0:HD + 1,
                                           h * QCW + ja * ST:
                                           h * QCW + (jb + 1) * ST],
                                        vb[:, k, 2 * g + h, :],
                                        pt[:, h * QCW + ja * ST:
                                           h * QCW + (jb + 1) * ST],
                                        start=(fast and idx == 0),
                                        stop=(fast and idx == nks - 1),
                                        skip_group_check=True)

                        pending = None
                        for idx, k in enumerate(ks):
                            runs = _runs(vjm[k])
                            psc = ppsc.tile([ST, 2 * QCW], F32, tag="psc")
                            for ja, jb in runs:
                                for h in range(2):
                                    nc.tensor.matmul(
                                        psc[:, h * QCW + ja * ST:
                                            h * QCW + (jb + 1) * ST],
                                        kt[g][h * HD:(h + 1) * HD,
                                              k * ST:(k + 1) * ST],
                                        qt[g][h * HD:(h + 1) * HD,
                                              qc * QCW + ja * ST:
                                              qc * QCW + (jb + 1) * ST],
                                        start=True, stop=True)
                            if pending is not None:
                                emit_av(*pending)
                            pt = ppt.tile([ST, 2 * QCW], BF16, tag="pt")
                            nc.scalar.activation(
                                pt[:, :], psc[:, :],
                                mybir.ActivationFunctionType.Exp,
                                scale=1.0 / float(np.sqrt(HD)))
                            for j in vjm[k]:
                                if cls[qc * QCT + j, k] == 2:
                                    m = mixed_idx[(qc * QCT + j, k)]
                                    for h in range(2):
                                        nc.vector.tensor_mul(
                                            pt[:, h * QCW + j * ST:
                                               h * QCW + (j + 1) * ST],
                                            pt[:, h * QCW + j * ST:
                                               h * QCW + (j + 1) * ST],
                                            mtb[:, m, :])
                            pending = (k, idx, runs, pt)
                            fil.pull(1)
                        emit_av(*pending)
                        # finalize (g, qc): free po via a DVE copy, then
                        # normalize out of SBUF.
                        sfin = pfin.tile([HD + 1, 2 * QCW], F32, tag="sfin")
                        nc.vector.tensor_copy(sfin[:, :], po[:, :])
                        ltmp = pfin.tile([1, 2 * QCW], F32, tag="ltmp")
                        nc.vector.tensor_copy(ltmp[:, :], sfin[HD:HD + 1, :])
                        rec = pfin.tile([1, 2 * QCW], F32, tag="rec")
                        nc.vector.reciprocal_approx_fast(rec[:, :], ltmp[:, :])
                        rbs = pfin.tile([HD, 2 * QCW], F32, tag="rbs")
                        nc.gpsimd.partition_broadcast(
                            rbs[:, :], rec[0:1, :], channels=HD)
                        for h in range(2):
                            nc.vector.tensor_mul(
                                cat[g][h * HD:(h + 1) * HD,
                                       qc * QCW:(qc + 1) * QCW],
                                sfin[0:HD, h * QCW:(h + 1) * QCW],
                                rbs[:, h * QCW:(h + 1) * QCW])
                        if g == NG - 1:
                            fil.open_gate(f"cat_qc{qc}")
                fil.drain_all()

    nc.compile()
    return nc


_CACHE = {}
LAST_RESULT = None


def _get_program(mask):
    key = mask.tobytes()
    if key not in _CACHE:
        cls, mixed = _classify(mask)
        _CACHE[key] = (_build(cls, mixed, max(len(mixed), 1)), cls, mixed)
    return _CACHE[key]


def kernel(x, mask, Wq, bq, Wk, bk, Wv, bv, Wo, bo):
    x = np.asarray(x, dtype=np.float32)
    mask = np.asarray(mask)
    Wq = np.asarray(Wq, dtype=np.float32)
    Wk = np.asarray(Wk, dtype=np.float32)
    Wv = np.asarray(Wv, dtype=np.float32)
    Wo = np.asarray(Wo, dtype=np.float32)
    nc, cls, mixed = _get_program(mask)

    n_maskt = max(len(mixed), 1)
    mt = np.zeros((n_maskt, ST, ST), dtype=BF)
    for i, (j, k) in enumerate(mixed):
        blk = mask[j * ST:(j + 1) * ST, k * ST:(k + 1) * ST]
        mt[i] = (blk != 0).T.astype(BF)

    def pack_pair(W, hh):
        # [128, NDC, NG, 128]: [p, c, g, m*64+e] = W[8hh + 2g+m, 128c+p, e]
        Wl = W[hh * HL:(hh + 1) * HL].reshape(NG, 2, NDC, ST, HD)
        return np.ascontiguousarray(
            Wl.transpose(3, 2, 0, 1, 4).reshape(ST, NDC, NG, ST).astype(BF))

    in_maps = []
    for c in range(N_CORES):
        b, hh = c // 2, c % 2
        Wvl = Wv[hh * HL:(hh + 1) * HL].reshape(HL, NDC, ST, HD)
        wvb = np.ascontiguousarray(
            Wvl.transpose(2, 1, 0, 3).reshape(ST, NDC, HL * HD).astype(BF))
        Wol = Wo[hh * HL * HD:(hh + 1) * HL * HD].reshape(NG, 2, HD, D)
        wob = np.ascontiguousarray(
            Wol.transpose(1, 2, 0, 3).reshape(ST, NG, D).astype(BF))
        bql = np.asarray(bq, dtype=np.float32)[hh * HL:(hh + 1) * HL]
        bkl = np.asarray(bk, dtype=np.float32)[hh * HL:(hh + 1) * HL]
        bqp = np.ascontiguousarray(
            bql.reshape(NG, 2, HD).transpose(1, 2, 0).reshape(ST, NG))
        bkp = np.ascontiguousarray(
            bkl.reshape(NG, 2, HD).transpose(1, 2, 0).reshape(ST, NG))
        m = {
            "x": np.ascontiguousarray(x[b].astype(BF)),
            "wqp": pack_pair(Wq, hh),
            "wkp": pack_pair(Wk, hh),
            "wvb": wvb,
            "wob": wob,
            "bqp": bqp,
            "bkp": bkp,
            "bvf": np.ascontiguousarray(
                np.asarray(bv, dtype=np.float32)[hh * HL:(hh + 1) * HL]),
            "bob": (np.asarray(bo, dtype=np.float32) if hh == 0
                    else np.zeros(D, dtype=np.float32)),
            "maskt": mt,
        }
        in_maps.append(m)

    res = run_bass_kernel_spmd(
        nc, in_maps, core_ids=list(range(N_CORES)),
        trace=os.environ.get("BASS_KERNEL_TRACE", "0") == "1")
    global LAST_RESULT
    LAST_RESULT = res

    out = np.empty((B, S, D), dtype=np.float32)
    for b in range(B):
        out[b] = (res.results[2 * b]["out"].astype(np.float32)
                  + res.results[2 * b + 1]["out"].astype(np.float32))
    return out
